# revision 1
# baseline (speedup 1.0000x reference)
"""BlockGRU Trainium2 kernel.

Block-diagonal GRU cell: 8 independent blocks (block_size 256), batch 2048,
input_dim 1024. Sharded one block per NeuronCore (8 cores).

Per-core layout: gates on partitions, batch on the free dimension
(everything transposed on the host, which is free). Matmul operands are
fp16 (measured end-to-end rel-L2 error vs the fp32 reference: 2.6e-4;
fp16 halves the DMA streams and runs the PE at full rate with fast
weight load); accumulation and all elementwise math stay fp32. r/z gate
pre-activations accumulate input-projection + hidden-projection directly
in PSUM; i_n and h_n are kept separate for the r-gating. Per-partition
biases fuse into ScalarE activation ops (sigmoid/tanh) and a
scalar_tensor_tensor on VectorE; 1-z runs on the idle GPSIMD engine.
"""

import sys

if "/opt/trn_rl_repo" not in sys.path:
    sys.path.insert(0, "/opt/trn_rl_repo")

import numpy as np

INPUT_DIM = 1024
HIDDEN_DIM = 2048
NUM_BLOCKS = 8
BS = HIDDEN_DIM // NUM_BLOCKS  # 256
G3 = 3 * BS                    # 768
BATCH = 2048
CHUNKS = [512, 512, 512, 256, 256]   # batch chunks (PSUM bank = 512 fp32;
                                     # small tail chunks shorten the post-PE tail)
KX = INPUT_DIM // 128          # 8 contraction tiles on the input side
KH = BS // 128                 # 2 contraction tiles on the hidden side
ST = BS // 128                 # 2 state partition-tiles per block

_cached = None


def _build():
    import concourse.tile as tile
    import concourse.mybir as mybir
    from concourse import bacc

    f32 = mybir.dt.float32
    f16 = mybir.dt.float16
    ALU = mybir.AluOpType
    ACT = mybir.ActivationFunctionType

    nc = bacc.Bacc("TRN2", target_bir_lowering=False, debug=False, num_devices=8)

    xT = nc.dram_tensor("xT", [INPUT_DIM, BATCH], f16, kind="ExternalInput")
    wih = nc.dram_tensor("wih", [INPUT_DIM, G3], f16, kind="ExternalInput")
    whh = nc.dram_tensor("whh", [BS, G3], f16, kind="ExternalInput")
    hT = nc.dram_tensor("hT", [BS, BATCH], f16, kind="ExternalInput")
    bias = nc.dram_tensor("bias", [128, 5 * ST], f32, kind="ExternalInput")
    oT = nc.dram_tensor("oT", [BS, BATCH], f32, kind="ExternalOutput")

    with tile.TileContext(nc) as tc:
        with (
            tc.tile_pool(name="const", bufs=1) as cp,
            tc.tile_pool(name="xin", bufs=3) as xp,
            tc.tile_pool(name="hin", bufs=3) as hp,
            tc.tile_pool(name="gates", bufs=4) as gp,
            tc.tile_pool(name="outs", bufs=3) as op,
            tc.tile_pool(name="psum", bufs=1, space="PSUM") as pp,
        ):
            # PE warm-up: harmless matmuls on a zeroed tile while the prefill
            # DMA runs, so the clock ramp (cold -> full rate) completes before
            # real work arrives. Uses the p0 PSUM slot ahead of chunk 0.
            wu = cp.tile([128, 32], f16, tag="wu")
            nc.vector.memset(wu[:], 0.0)
            pdummy = pp.tile([128, 32], f32, tag="p0", name="pdummy")
            for _ in range(48):
                nc.tensor.matmul(pdummy[0:32, :], wu[:, 0:32], wu[:],
                                 start=True, stop=True)

            # --- DMA prologue. The DMA queue is serial at HBM bandwidth, so
            # emission order == arrival order == PE consumption order: the
            # x-side weights and chunk-0 columns first (bulk of PE work),
            # hidden-side weights/state + biases after (consumed at the end
            # of chunk 0's accumulation). ---
            c0w = CHUNKS[0]
            cs0 = slice(0, c0w)
            wih_sb = []
            x0_t = []
            # k-tiles load pairwise-merged via 3D access patterns: fewer
            # DMA descriptors means the serial DMA stream outpaces PE's
            # k-major consumption, so chunk 0 runs stall-free.
            for kp in range(0, KX, 2):
                wm = cp.tile([128, 2 * G3], f16, tag=f"wih{kp}", name=f"wihm{kp}")
                nc.sync.dma_start(
                    wm[:].rearrange("p (k g) -> p k g", k=2),
                    wih.ap()[kp * 128:(kp + 2) * 128, :]
                        .rearrange("(k p) g -> p k g", p=128))
                wih_sb.append(wm[:, 0:G3])
                wih_sb.append(wm[:, G3:2 * G3])
                xm = xp.tile([128, 2 * c0w], f16, tag=f"x{kp}", name=f"xm{kp}")
                nc.sync.dma_start(
                    xm[:].rearrange("p (k c) -> p k c", k=2),
                    xT.ap()[kp * 128:(kp + 2) * 128, cs0]
                        .rearrange("(k p) b -> p k b", p=128))
                x0_t.append(xm[:, 0:c0w])
                x0_t.append(xm[:, c0w:2 * c0w])
                if kp == 4:
                    bias_sb = cp.tile([128, 5 * ST], f32, tag="bias")
                    nc.sync.dma_start(bias_sb[:], bias.ap())
            brz_sb = bias_sb[:, 0:2 * ST]
            bzn_sb = bias_sb[:, 2 * ST:3 * ST]
            bin_sb = bias_sb[:, 3 * ST:4 * ST]
            bhn_sb = bias_sb[:, 4 * ST:5 * ST]
            whm = cp.tile([128, 2 * G3], f16, tag="whm")
            nc.sync.dma_start(
                whm[:].rearrange("p (k g) -> p k g", k=2),
                whh.ap().rearrange("(k p) g -> p k g", p=128))
            whh_sb = [whm[:, 0:G3], whm[:, G3:2 * G3]]
            h0m = hp.tile([128, 2 * c0w], f16, tag="h0m")
            nc.sync.dma_start(
                h0m[:].rearrange("p (k c) -> p k c", k=2),
                hT.ap()[:, cs0].rearrange("(k p) b -> p k b", p=128))
            h0_t = [h0m[:, 0:c0w], h0m[:, c0w:2 * c0w]]

            cstart = 0
            for c, cw in enumerate(CHUNKS):
                cs = slice(cstart, cstart + cw)
                cstart += cw
                if c == 0:
                    x_t, h_t = x0_t, h0_t
                else:
                    x_t = []
                    for kp in range(0, KX, 2):
                        xm2 = xp.tile([128, 2 * cw], f16, tag=f"x{kp}",
                                      name=f"xc{kp}")
                        nc.sync.dma_start(
                            xm2[:].rearrange("p (k c) -> p k c", k=2),
                            xT.ap()[kp * 128:(kp + 2) * 128, cs]
                                .rearrange("(k p) b -> p k b", p=128))
                        x_t.append(xm2[:, 0:cw])
                        x_t.append(xm2[:, cw:2 * cw])
                    hm2 = hp.tile([128, 2 * cw], f16, tag="h0m", name="hc")
                    nc.sync.dma_start(
                        hm2[:].rearrange("p (k c) -> p k c", k=2),
                        hT.ap()[:, cs].rearrange("(k p) b -> p k b", p=128))
                    h_t = [hm2[:, 0:cw], hm2[:, cw:2 * cw]]

                # PSUM accumulators. r/z gates take input-proj + hidden-proj
                # into the same bank (only their sum is needed downstream).
                p_rz = [pp.tile([128, cw], f32, tag=f"p{gt}", name=f"prz{gt}")
                        for gt in range(2 * ST)]
                p_in = [pp.tile([128, cw], f32, tag=f"p{2 * ST + t_}", name=f"pin{t_}")
                        for t_ in range(ST)]
                p_hn = [pp.tile([128, cw], f32, tag=f"p{3 * ST + t_}", name=f"phn{t_}")
                        for t_ in range(ST)]

                # Input-side first, k-major, so PE consumption tracks the DMA
                # arrival order (wih[k]/x[k] pairs).  The last x k-tile plus
                # all hidden-side matmuls form per-psum "tail groups" ordered
                # so psums complete staggered: r-gates first (sigmoids start
                # draining banks early), i_n last (shortest post-PE chain).
                def gsl(gt):
                    return slice(gt * 128, (gt + 1) * 128)

                for k in range(KX - 1):
                    for gt in range(2 * ST):
                        nc.tensor.matmul(p_rz[gt][:], wih_sb[k][:, gsl(gt)],
                                         x_t[k][:], start=(k == 0), stop=False)
                    for t_ in range(ST):
                        nc.tensor.matmul(p_in[t_][:], wih_sb[k][:, gsl(4 + t_)],
                                         x_t[k][:], start=(k == 0), stop=False)
                kl = KX - 1
                last = (c == len(CHUNKS) - 1)
                o = op.tile([128, ST * cw], f32, tag="o")

                def r_tail(t_):
                    nc.tensor.matmul(p_rz[t_][:], wih_sb[kl][:, gsl(t_)],
                                     x_t[kl][:], start=False, stop=False)
                    for k in range(KH):
                        nc.tensor.matmul(p_rz[t_][:], whh_sb[k][:, gsl(t_)],
                                         h_t[k][:], start=False, stop=(k == KH - 1))

                def hn_tail(t_):
                    for k in range(KH):
                        nc.tensor.matmul(p_hn[t_][:], whh_sb[k][:, gsl(4 + t_)],
                                         h_t[k][:], start=(k == 0), stop=(k == KH - 1))

                def in_tail(t_):
                    nc.tensor.matmul(p_in[t_][:], wih_sb[kl][:, gsl(4 + t_)],
                                     x_t[kl][:], start=False, stop=True)

                def z_tail(t_):
                    gt = ST + t_
                    nc.tensor.matmul(p_rz[gt][:], wih_sb[kl][:, gsl(gt)],
                                     x_t[kl][:], start=False, stop=False)
                    for k in range(KH):
                        nc.tensor.matmul(p_rz[gt][:], whh_sb[k][:, gsl(gt)],
                                         h_t[k][:], start=False, stop=(k == KH - 1))

                def ew_r(t_):
                    r = gp.tile([128, cw], f32, tag=f"r{t_}", name=f"r{t_}")
                    nc.scalar.activation(r[:], p_rz[t_][:], ACT.Sigmoid,
                                         bias=brz_sb[:, t_:t_ + 1])
                    a = gp.tile([128, cw], f32, tag=f"a{t_}", name=f"a{t_}")
                    nc.vector.scalar_tensor_tensor(
                        a[:], p_hn[t_][:], bhn_sb[:, t_:t_ + 1], r[:],
                        ALU.add, ALU.mult)
                    return a

                def ew_z(t_):
                    z = gp.tile([128, cw], f32, tag=f"z{t_}", name=f"z{t_}")
                    nc.scalar.activation(z[:], p_rz[ST + t_][:], ACT.Sigmoid,
                                         bias=brz_sb[:, ST + t_:ST + t_ + 1])
                    zc = gp.tile([128, cw], f32, tag=f"zc{t_}", name=f"zc{t_}")
                    nc.gpsimd.tensor_scalar(zc[:], z[:], -1.0, 1.0,
                                            ALU.mult, ALU.add)
                    return z, zc

                def ew_zh(t_, z):
                    zh = gp.tile([128, cw], f32, tag=f"zh{t_}", name=f"zh{t_}")
                    nc.vector.tensor_mul(zh[:], z[:], h_t[t_][:])
                    return zh

                def ew_tanh(t_, a):
                    b2 = gp.tile([128, cw], f32, tag=f"b{t_}", name=f"b{t_}")
                    nc.vector.tensor_add(b2[:], a[:], p_in[t_][:])
                    n_ = gp.tile([128, cw], f32, tag=f"n{t_}", name=f"n{t_}")
                    nc.scalar.activation(n_[:], b2[:], ACT.Tanh,
                                         bias=bin_sb[:, t_:t_ + 1])
                    return n_

                def ew_out(t_, n_, zc, zh):
                    e = gp.tile([128, cw], f32, tag=f"e{t_}", name=f"e{t_}")
                    nc.vector.tensor_mul(e[:], n_[:], zc[:])
                    nc.vector.tensor_add(o[:, t_ * cw:(t_ + 1) * cw], e[:],
                                         zh[:])

                if not last:
                    # staggered psum completion: r-gates first (sigmoids free
                    # banks for the next chunk), i_n last (short post chain)
                    for t_ in range(ST):
                        r_tail(t_)
                    for t_ in range(ST):
                        hn_tail(t_)
                    for t_ in range(ST):
                        z_tail(t_)
                    for t_ in range(ST):
                        in_tail(t_)
                    as_ = [ew_r(t_) for t_ in range(ST)]
                    zzc = [ew_z(t_) for t_ in range(ST)]
                    zhs = [ew_zh(t_, zzc[t_][0]) for t_ in range(ST)]
                    ns_ = [ew_tanh(t_, as_[t_]) for t_ in range(ST)]
                    for t_ in range(ST):
                        ew_out(t_, ns_[t_], zzc[t_][1], zhs[t_])
                    nc.scalar.dma_start(
                        oT.ap().rearrange("(t p) b -> p t b", p=128)[:, :, cs],
                        o[:].rearrange("p (t c) -> p t c", t=ST))
                else:
                    # final chunk: i_n psums complete before the z-gates so
                    # the b2/tanh chain runs under the last matmuls; b2 goes
                    # ahead of zh on the VectorE queue; per-tile output DMAs
                    # on the scalar and sync DGE queues.
                    for t_ in range(ST):
                        r_tail(t_)
                    for t_ in range(ST):
                        hn_tail(t_)
                    for t_ in range(ST):
                        in_tail(t_)
                    for t_ in range(ST):
                        z_tail(t_)
                    as_ = [ew_r(t_) for t_ in range(ST)]
                    zzc = [ew_z(t_) for t_ in range(ST)]
                    ns_ = [ew_tanh(t_, as_[t_]) for t_ in range(ST)]
                    zhs = [ew_zh(t_, zzc[t_][0]) for t_ in range(ST)]
                    for t_ in range(ST):
                        ew_out(t_, ns_[t_], zzc[t_][1], zhs[t_])
                        eng = nc.scalar if t_ == 0 else nc.sync
                        eng.dma_start(
                            oT.ap()[t_ * 128:(t_ + 1) * 128, cs],
                            o[:, t_ * cw:(t_ + 1) * cw])

    nc.compile()
    return nc


def _get_nc():
    global _cached
    if _cached is None:
        _cached = _build()
    return _cached


def kernel(input, hidden, W_ih, W_hh, b_ih, b_hh):
    input = np.asarray(input, dtype=np.float32)
    hidden = np.asarray(hidden, dtype=np.float32)
    W_ih = np.asarray(W_ih, dtype=np.float32)
    W_hh = np.asarray(W_hh, dtype=np.float32)
    b_ih = np.asarray(b_ih, dtype=np.float32)
    b_hh = np.asarray(b_hh, dtype=np.float32)

    nc = _get_nc()
    from concourse.bass_utils import run_bass_kernel_spmd

    xT = np.ascontiguousarray(input.T.astype(np.float16))
    in_maps = []
    for n in range(NUM_BLOCKS):
        brz_n = (b_ih[n, :2 * BS] + b_hh[n, :2 * BS]).reshape(2 * ST, 128).T
        bzn_n = -brz_n[:, ST:]
        bin_n = b_ih[n, 2 * BS:].reshape(ST, 128).T
        bhn_n = b_hh[n, 2 * BS:].reshape(ST, 128).T
        bias_n = np.concatenate([brz_n, bzn_n, bin_n, bhn_n], axis=1)
        in_maps.append({
            "xT": xT,
            "wih": np.ascontiguousarray(W_ih[n].T.astype(np.float16)),
            "whh": np.ascontiguousarray(W_hh[n].T.astype(np.float16)),
            "hT": np.ascontiguousarray(hidden[:, n * BS:(n + 1) * BS].T.astype(np.float16)),
            "bias": np.ascontiguousarray(bias_n),
        })

    res = run_bass_kernel_spmd(nc, in_maps, core_ids=list(range(NUM_BLOCKS)))
    out = np.empty((BATCH, HIDDEN_DIM), dtype=np.float32)
    for n in range(NUM_BLOCKS):
        out[:, n * BS:(n + 1) * BS] = res.results[n]["oT"].T
    return out



# revision 78
# speedup vs baseline: 1.4660x; 1.4660x over previous
"""BlockGRU Trainium2 kernel — fp8 DoubleRow edition.

Block-diagonal GRU cell: 8 independent blocks (block_size 256), batch 2048,
input_dim 1024. Sharded one block per NeuronCore.

Core idea: all matmuls run as fp8(e4m3) DoubleRow — 2 contraction k-tiles
(256 dims) per instruction at 0.5 cycles/row, 4x the fp16 PE throughput.
Raw e4m3 quantization noise fails the 2e-2 gate (rel-L2 2.6e-2), so the
noise-dominant paths get cheap fp8 correction passes (validated by host-side
simulation, rel-L2 1.3e-2):
  - z & n gates: + x_lo @ (W8/16), where x_lo = q8(16*(xc - x_hi)) recovers
    the input quantization residual (scaled into e4m3's normal range).
  - n gate:      + x_hi @ q8(16*E)/16, where E = 32W - W8 is the weight
    quantization residual.
All biases for the x-side are folded into the input: xc = x + c with
W8^T c = 32b (min-norm lstsq), so sigmoid/tanh run bias-free and the
r0/r1/z0/z1 pre-activations drain in a single merged 4-bank ACT instruction.
The hidden-side n bias rides the Pool-engine scalar_tensor_tensor.

Blend is out = n + z*(h16 - n) in fp16 (DVE 2x mode); output written fp16
and upcast on host.
"""

import sys

if "/opt/trn_rl_repo" not in sys.path:
    sys.path.insert(0, "/opt/trn_rl_repo")

import numpy as np
import ml_dtypes

F8 = ml_dtypes.float8_e4m3

INPUT_DIM = 1024
HIDDEN_DIM = 2048
NUM_BLOCKS = 8
BS = HIDDEN_DIM // NUM_BLOCKS      # 256
G3 = 3 * BS                        # 768
BATCH = 2048
CW = 256                           # compute chunk (psum fp32 half-bank)
NCH = BATCH // CW                  # 8 compute chunks
NG = NCH // 2                      # 4 dma/elementwise groups of 512
KX = INPUT_DIM // 128              # 8 x k-tiles (4 DR pairs)
KIN = 2 * KX + 2                   # xin k-tiles: x_hi(8) + x_lo(8) + h8(2)

_cached = None


def _build():
    import concourse.tile as tile
    import concourse.mybir as mybir
    from concourse import bacc

    f32 = mybir.dt.float32
    f16 = mybir.dt.float16
    f8 = mybir.dt.float8e4
    ALU = mybir.AluOpType
    ACT = mybir.ActivationFunctionType
    DR = mybir.MatmulPerfMode.DoubleRow

    nc = bacc.Bacc("TRN2", target_bir_lowering=False, debug=False, num_devices=8)

    # xin rows: x_hi (1024, k-major) ++ x_lo (1024) ++ h8 (256); cols batch
    xin = nc.dram_tensor("xin", [128 * KIN, BATCH], f8, kind="ExternalInput")
    # wpk rows: W8_ih (1024 k-major) ++ W8_hh (256); cols gates r|z|n
    wpk = nc.dram_tensor("wpk", [INPUT_DIM + BS, G3], f8, kind="ExternalInput")
    # wcor: q8(16*(32W_ih - W8_ih))[:, n]/16 — the n-gate weight-residual
    # term. (The W8/16 tensors for the x_lo passes are derived on-chip.)
    wcor = nc.dram_tensor("wcor", [INPUT_DIM, BS], f8, kind="ExternalInput")
    # q8(32*b_hh_n) packed [1, 256] — injected into P_hn via a ones-matmul
    bim = nc.dram_tensor("bim", [1, 256], f8, kind="ExternalInput")
    h16 = nc.dram_tensor("h16", [BS, BATCH], f16, kind="ExternalInput")
    oT = nc.dram_tensor("oT", [BS, BATCH], f16, kind="ExternalOutput")

    with tile.TileContext(nc) as tc:
        with (
            tc.tile_pool(name="const", bufs=1) as cp,
            tc.tile_pool(name="xin_p", bufs=3) as xp,
            tc.tile_pool(name="h16_p", bufs=2) as hp,
            tc.tile_pool(name="work", bufs=2) as gp,
            tc.tile_pool(name="psum", bufs=1, space="PSUM") as pp,
        ):
            # PE warm-up: keep the PE continuously busy through the DMA
            # prologue so the p-state ramp (cold -> 2.4GHz after 3us) is done
            # before real matmuls issue.
            wu = cp.tile([128, 32], f8, tag="wu")
            nc.vector.memset(wu[:], 0.0)
            pdummy = pp.tile([128, 2048], f32, tag="rz", name="pdummy")
            for i in range(110):
                nc.tensor.matmul(pdummy[0:32, 0:32], wu[:, 0:32], wu[:],
                                 start=True, stop=True)

            # --- DMA prologue (sync/SP queue, serial in program order).
            # Finely split at the head so chunk-0 matmuls start as early as
            # possible; the emission of compute below is kp-outer to match
            # this arrival order.
            wA = cp.tile([128, 4 * G3], f8, tag="wA")     # W8_ih k0..k3
            wAv = wA[:].rearrange("p (k g) -> p k g", k=4)
            g0 = xp.tile([128, KIN * 512], f8, tag="xg", name="xg0")
            g0v = g0[:].rearrange("p (k b) -> p k b", k=KIN)
            nc.sync.dma_start(
                wAv[:, 0:2, :],
                wpk.ap()[0:256, :].rearrange("(k p) g -> p k g", p=128))
            nc.sync.dma_start(
                g0v[:, 0:2, :],
                xin.ap()[0:256, 0:512].rearrange("(k p) b -> p k b", p=128))
            nc.sync.dma_start(
                wAv[:, 2:4, :],
                wpk.ap()[256:512, :].rearrange("(k p) g -> p k g", p=128))
            nc.sync.dma_start(
                g0v[:, 2:KX, :],
                xin.ap()[256:1024, 0:512].rearrange("(k p) b -> p k b", p=128))
            wB = cp.tile([128, 6 * G3], f8, tag="wB")     # k4..k7 + hh k0..k1
            wBv0 = wB[:].rearrange("p (k g) -> p k g", k=6)
            nc.sync.dma_start(
                wBv0[:, 0:2, :],
                wpk.ap()[512:768, :].rearrange("(k p) g -> p k g", p=128))
            nc.sync.dma_start(
                wBv0[:, 2:6, :],
                wpk.ap()[768:1280, :].rearrange("(k p) g -> p k g", p=128))
            # h8 ahead of x_lo: the h-side matmuls run before the x_lo passes
            nc.sync.dma_start(
                g0v[:, 2 * KX:KIN, :],
                xin.ap()[2048:128 * KIN, 0:512]
                    .rearrange("(k p) b -> p k b", p=128))
            nc.sync.dma_start(
                g0v[:, KX:2 * KX, :],
                xin.ap()[1024:2048, 0:512]
                    .rearrange("(k p) b -> p k b", p=128))
            # n-gate weight-residual correction weights
            we = cp.tile([128, KX * BS], f8, tag="we")
            nc.sync.dma_start(
                we[:].rearrange("p (k g) -> p k g", k=KX),
                wcor.ap().rearrange("(k p) g -> p k g", p=128))
            h0 = hp.tile([128, 2 * 512], f16, tag="hg", name="hg0")
            nc.sync.dma_start(
                h0[:].rearrange("p (t b) -> p t b", t=2),
                h16.ap()[:, 0:512].rearrange("(t p) b -> p t b", p=128))
            # bias row for the P_hn injection (partition 0 holds 32*bhn;
            # cols 256:512 stay zero and serve as the second DR k-tile)
            bim_sb = cp.tile([128, 512], f8, tag="bim")
            nc.vector.memset(bim_sb[:], 0.0)
            nc.sync.dma_start(bim_sb[0:1, 0:256], bim.ap())
            ones8 = cp.tile([128, 1024], f8, tag="ones8")
            nc.vector.memset(ones8[:], 1.0)

            # W8/16 for the x_lo correction passes, derived on-chip on the
            # (head-idle) DVE instead of spending DMA stream bytes; z columns
            # first since the z x_lo passes run before the i_n ones
            wl = cp.tile([128, KX * 2 * BS], f8, tag="wl")
            wBv = wBv0
            wlvd = wl[:].rearrange("p (k g) -> p k g", k=KX)
            nc.vector.tensor_scalar_mul(
                wlvd[:, 0:4, 0:BS], wAv[:, :, BS:2 * BS], 1.0 / 16)
            nc.vector.tensor_scalar_mul(
                wlvd[:, 4:KX, 0:BS], wBv[:, 0:4, BS:2 * BS], 1.0 / 16)
            nc.gpsimd.tensor_scalar_mul(
                wlvd[:, 0:4, BS:2 * BS], wAv[:, :, 2 * BS:G3], 1.0 / 16)
            nc.gpsimd.tensor_scalar_mul(
                wlvd[:, 4:KX, BS:2 * BS], wBv[:, 0:4, 2 * BS:G3], 1.0 / 16)

            # remaining group loads (x_hi part first), still on the SP queue
            # ahead of all output DMAs (SP SEQ is in-order; outputs must not
            # gate inputs)
            xgs, hgs = [None], [h0]
            for g in range(1, NG):
                xg = xp.tile([128, KIN * 512], f8, tag="xg", name=f"xg{g}")
                xgv_ = xg[:].rearrange("p (k b) -> p k b", k=KIN)
                nc.sync.dma_start(
                    xgv_[:, 0:KX, :],
                    xin.ap()[0:1024, g * 512:(g + 1) * 512]
                        .rearrange("(k p) b -> p k b", p=128))
                nc.sync.dma_start(
                    xgv_[:, KX:KIN, :],
                    xin.ap()[1024:128 * KIN, g * 512:(g + 1) * 512]
                        .rearrange("(k p) b -> p k b", p=128))
                xgs.append(xg)
                hg = hp.tile([128, 2 * 512], f16, tag="hg", name=f"hg{g}")
                nc.sync.dma_start(
                    hg[:].rearrange("p (t b) -> p t b", t=2),
                    h16.ap()[:, g * 512:(g + 1) * 512]
                        .rearrange("(t p) b -> p t b", p=128))
                hgs.append(hg)

            # weight views: [128, ktile, gate] f8
            wlv = wlvd
            wev = we[:].rearrange("p (k g) -> p k g", k=KX)

            def w_hi(kp, gs):
                # stationary [128, 2, 128] for x_hi k-pair kp, gate cols gs
                if kp < 2:
                    return wAv[:, 2 * kp:2 * kp + 2, gs]
                return wBv[:, 2 * kp - 4:2 * kp - 2, gs]

            def gsl(t):
                return slice(t * 128, (t + 1) * 128)

            # Chunks: [512, 512, 512, 256, 256] batch columns. Every psum
            # accumulator occupies its own 2KB bank (HW zeroes the whole
            # zero-region when an accumulation group starts), so wider chunks
            # mean fewer chunk-boundary serialization points. The short final
            # chunks keep the post-PE drain chain off the critical path.
            CHUNKS = [(0, 0, 512), (1, 0, 512), (2, 0, 512),
                      (3, 0, 256), (3, 256, 256)]

            def mm_rz_xhi(rz, xgv, cs, w, tiles=range(4), kp_outer=True):
                # kp-outer matches the k-major DMA arrival order
                order = ([(kp, t) for kp in range(4) for t in tiles]
                         if kp_outer else
                         [(kp, t) for t in tiles for kp in range(4)])
                for kp, t in order:
                    nc.tensor.matmul(
                        rz[:, t * 512:t * 512 + w], w_hi(kp, gsl(t)),
                        xgv[:, 2 * kp:2 * kp + 2, cs],
                        start=(kp == 0), stop=False, perf_mode=DR)

            def mm_in_xhi(nh, xgv, cs, w):
                for kp in range(4):
                    for t in range(2):
                        nc.tensor.matmul(
                            nh[:, t * 512:t * 512 + w], w_hi(kp, gsl(4 + t)),
                            xgv[:, 2 * kp:2 * kp + 2, cs],
                            start=(kp == 0), stop=False, perf_mode=DR)

            def mm_hside_rz(rz, xgv, cs, w, tiles, stop=True):
                for t in tiles:                 # h-side closes each rz tile
                    nc.tensor.matmul(
                        rz[:, t * 512:t * 512 + w], wBv[:, 4:6, gsl(t)],
                        xgv[:, 2 * KX:2 * KX + 2, cs],
                        start=False, stop=stop, perf_mode=DR)

            bimv = bim_sb[:].rearrange("p (i u) -> p i u", u=128)
            onesv = ones8[:].rearrange("p (k n) -> p k n", k=2)

            def mm_hn(nh, xgv, cs, w):
                for t in range(2):              # bias inject + h_n
                    nc.tensor.matmul(
                        nh[:, (2 + t) * 512:(2 + t) * 512 + w],
                        bimv[:, t:t + 3:2, :], onesv[:, :, 0:w],
                        start=True, stop=False, perf_mode=DR)
                    nc.tensor.matmul(
                        nh[:, (2 + t) * 512:(2 + t) * 512 + w],
                        wBv[:, 4:6, gsl(4 + t)],
                        xgv[:, 2 * KX:2 * KX + 2, cs],
                        start=False, stop=True, perf_mode=DR)

            def mm_zlo(rz, xgv, cs, w, stop=True):
                for t in range(2):              # z x_lo fix
                    for kp in range(4):
                        nc.tensor.matmul(
                            rz[:, (2 + t) * 512:(2 + t) * 512 + w],
                            wlv[:, 2 * kp:2 * kp + 2, gsl(t)],
                            xgv[:, KX + 2 * kp:KX + 2 * kp + 2, cs],
                            start=False, stop=stop and (kp == 3), perf_mode=DR)

            def mm_inlo(nh, xgv, cs, w):
                for t in range(2):              # i_n x_lo + W-residual fixes
                    for kp in range(4):
                        nc.tensor.matmul(
                            nh[:, t * 512:t * 512 + w],
                            wlv[:, 2 * kp:2 * kp + 2, gsl(2 + t)],
                            xgv[:, KX + 2 * kp:KX + 2 * kp + 2, cs],
                            start=False, stop=False, perf_mode=DR)
                    for kp in range(4):
                        nc.tensor.matmul(
                            nh[:, t * 512:t * 512 + w],
                            wev[:, 2 * kp:2 * kp + 2, gsl(t)],
                            xgv[:, 2 * kp:2 * kp + 2, cs],
                            start=False, stop=(kp == 3), perf_mode=DR)

            def sigmoid_part(rz, rzc, lo, hi, w):
                nc.scalar.activation(
                    rzc[:].rearrange("p (s c) -> p s c", c=512)[:, lo:hi, 0:w],
                    rz[:].rearrange("p (s c) -> p s c", c=512)[:, lo:hi, 0:w],
                    ACT.Sigmoid, scale=1.0 / 32)

            # Software-pipelined post-PE stream: chunk c's sigmoid/a/b2 are
            # emitted with chunk c, but its tanh + blend + output DMA are
            # emitted during chunk c+1, keeping the in-order ACT and DVE
            # queues from stalling on each other's round-trips.
            pending = []   # [(rzc, b2, hg, cstart, goff, w, par)]

            def emit_ab2(nh, rzc, par, ci, w):
                # a = P_hn * r (bias pre-injected; DVE — GPSIMD can't do PSUM)
                at = gp.tile([128, 1024], f32, tag=f"a{par}", name=f"a{ci}")
                nhv = nh[:].rearrange("p (s c) -> p s c", c=512)
                atv = at[:].rearrange("p (t c) -> p t c", t=2)
                rzcv = rzc[:].rearrange("p (s c) -> p s c", c=512)
                nc.vector.tensor_mul(atv[:, :, 0:w], nhv[:, 2:4, 0:w],
                                     rzcv[:, 0:2, 0:w])
                b2 = gp.tile([128, 1024], f32, tag=f"b2{par}", name=f"b2{ci}")
                nc.vector.tensor_add(
                    b2[:].rearrange("p (t c) -> p t c", t=2)[:, :, 0:w],
                    atv[:, :, 0:w], nhv[:, 0:2, 0:w])
                return b2

            def flush_pending(last=False):
                if not pending:
                    return
                rzc, b2, hg_, cstart, goff, w, par, ci = pending.pop()
                gsb = slice(goff, goff + w)
                n_t = gp.tile([128, 1024], f16, tag=f"n{par}", name=f"n{ci}")
                ntv = n_t[:].rearrange("p (t c) -> p t c", t=2)
                nc.scalar.activation(
                    ntv[:, :, 0:w],
                    b2[:].rearrange("p (t c) -> p t c", t=2)[:, :, 0:w],
                    ACT.Tanh, scale=1.0 / 32)
                d_t = gp.tile([128, 1024], f16, tag=f"d{par}", name=f"d{ci}")
                dtv = d_t[:].rearrange("p (t c) -> p t c", t=2)
                # steady-state d runs on the otherwise-idle Pool engine
                # (SBUF-only there); the final chunk keeps DVE for latency
                eng = nc.vector
                eng.tensor_sub(
                    dtv[:, :, 0:w],
                    hg_[:].rearrange("p (t b) -> p t b", t=2)[:, :, gsb],
                    ntv[:, :, 0:w])
                m_t = gp.tile([128, 1024], f16, tag=f"m{par}", name=f"m{ci}")
                mtv = m_t[:].rearrange("p (t c) -> p t c", t=2)
                nc.vector.tensor_mul(
                    mtv[:, :, 0:w],
                    rzc[:].rearrange("p (s c) -> p s c", c=512)[:, 2:4, 0:w],
                    dtv[:, :, 0:w])
                o_t = gp.tile([128, 1024], f16, tag=f"o{par}", name=f"o{ci}")
                otv = o_t[:].rearrange("p (t c) -> p t c", t=2)
                nc.vector.tensor_add(otv[:, :, 0:w], ntv[:, :, 0:w],
                                     mtv[:, :, 0:w])
                nc.sync.dma_start(
                    oT.ap()[:, cstart:cstart + w]
                        .rearrange("(t p) c -> p t c", p=128),
                    otv[:, :, 0:w])

            def chunk_tail(nh, rzc, hg_, cstart, goff, w, par, ci):
                flush_pending()
                b2 = emit_ab2(nh, rzc, par, ci, w)
                pending.append((rzc, b2, hg_, cstart, goff, w, par, ci))

            for ci, (g, goff, w) in enumerate(CHUNKS):
                if g == 0:
                    xgv, hg = g0v, h0
                else:
                    xgv = xgs[g][:].rearrange("p (k b) -> p k b", k=KIN)
                    hg = hgs[g]
                par = "AB"[ci % 2]
                cs = slice(goff, goff + w)
                cstart = g * 512 + goff
                rz = pp.tile([128, 2048], f32, tag="rz", name=f"rz{ci}")
                nh = pp.tile([128, 2048], f32, tag="nh", name=f"nh{ci}")
                rzc = gp.tile([128, 2048], f16, tag=f"rzc{par}",
                              name=f"rzc{ci}")

                if g == 0:
                    # chunk 0 is DMA-starved: z's x_lo pass (whose data
                    # arrives last) goes at the end, with the i_n x_hi wave
                    # filling the gap
                    mm_rz_xhi(rz, xgv, cs, w, tiles=(2, 3, 0, 1))
                    mm_hside_rz(rz, xgv, cs, w, tiles=(2, 3), stop=False)
                    mm_hside_rz(rz, xgv, cs, w, tiles=(0, 1))
                    sigmoid_part(rz, rzc, 0, 2, w)
                    mm_hn(nh, xgv, cs, w)
                    mm_in_xhi(nh, xgv, cs, w)
                    mm_zlo(rz, xgv, cs, w, stop=True)
                    sigmoid_part(rz, rzc, 2, 4, w)
                    mm_inlo(nh, xgv, cs, w)
                else:
                    mm_rz_xhi(rz, xgv, cs, w)
                    mm_zlo(rz, xgv, cs, w, stop=False)
                    mm_hside_rz(rz, xgv, cs, w, tiles=(2, 3))
                    mm_hside_rz(rz, xgv, cs, w, tiles=(0, 1))
                    mm_hn(nh, xgv, cs, w)
                    sigmoid_part(rz, rzc, 0, 4, w)
                    mm_in_xhi(nh, xgv, cs, w)
                    mm_inlo(nh, xgv, cs, w)
                chunk_tail(nh, rzc, hg, cstart, goff, w, par, ci)

            flush_pending(last=True)

    nc.compile()
    return nc


def _get_nc():
    global _cached
    if _cached is None:
        _cached = _build()
    return _cached


def _q8(v):
    return np.asarray(v, np.float32).astype(F8)


def kernel(input, hidden, W_ih, W_hh, b_ih, b_hh):
    input = np.asarray(input, dtype=np.float32)
    hidden = np.asarray(hidden, dtype=np.float32)
    W_ih = np.asarray(W_ih, dtype=np.float32)
    W_hh = np.asarray(W_hh, dtype=np.float32)
    b_ih = np.asarray(b_ih, dtype=np.float32)
    b_hh = np.asarray(b_hh, dtype=np.float32)

    nc = _get_nc()
    from concourse.bass_utils import run_bass_kernel_spmd

    in_maps = []
    for n in range(NUM_BLOCKS):
        WiT = 32.0 * W_ih[n].T                    # [1024, 768] fp32
        WhT = 32.0 * W_hh[n].T                    # [256, 768]
        Wi8 = _q8(WiT)
        Wh8 = _q8(WhT)
        Wi8f = Wi8.astype(np.float32)
        wpk = np.concatenate([Wi8, Wh8], axis=0)  # fp8 [1280, 768]
        E = _q8(16.0 * (WiT - Wi8f)).astype(np.float32)
        wcor = (E[:, 2 * BS:] / 16.0).astype(F8)  # [1024, 256]

        brz = b_ih[n, :2 * BS] + b_hh[n, :2 * BS]
        bin_ = b_ih[n, 2 * BS:]
        bhn = b_hh[n, 2 * BS:]
        b_vec = np.concatenate([brz, bin_]) * 32.0
        c, *_ = np.linalg.lstsq(Wi8f.T, b_vec, rcond=None)

        xc = input + c[None, :]
        x_hi = _q8(xc)
        x_lo = _q8(16.0 * (xc - x_hi.astype(np.float32)))
        hb = hidden[:, n * BS:(n + 1) * BS]
        h8 = _q8(hb)
        xin = np.concatenate(
            [np.ascontiguousarray(x_hi.T), np.ascontiguousarray(x_lo.T),
             np.ascontiguousarray(h8.T)], axis=0)  # [2304, 2048] fp8

        bim_t = np.ascontiguousarray(
            (32.0 * bhn).reshape(1, 256)).astype(F8)

        in_maps.append({
            "xin": xin,
            "wpk": np.ascontiguousarray(wpk),
            "wcor": np.ascontiguousarray(wcor),
            "bim": bim_t,
            "h16": np.ascontiguousarray(hb.T.astype(np.float16)),
        })

    res = run_bass_kernel_spmd(nc, in_maps, core_ids=list(range(NUM_BLOCKS)))
    out = np.empty((BATCH, HIDDEN_DIM), dtype=np.float32)
    for n in range(NUM_BLOCKS):
        out[:, n * BS:(n + 1) * BS] = res.results[n]["oT"].T.astype(np.float32)
    return out


# revision 79
# speedup vs baseline: 1.4967x; 1.0210x over previous
"""BlockGRU Trainium2 kernel — fp8 DoubleRow edition.

Block-diagonal GRU cell: 8 independent blocks (block_size 256), batch 2048,
input_dim 1024. Sharded one block per NeuronCore.

Core idea: all matmuls run as fp8(e4m3) DoubleRow — 2 contraction k-tiles
(256 dims) per instruction at 0.5 cycles/row, 4x the fp16 PE throughput.
Raw e4m3 quantization noise fails the 2e-2 gate (rel-L2 2.6e-2), so the
noise-dominant paths get cheap fp8 correction passes (validated by host-side
simulation, rel-L2 1.3e-2):
  - z & n gates: + x_lo @ (W8/16), where x_lo = q8(16*(xc - x_hi)) recovers
    the input quantization residual (scaled into e4m3's normal range).
  - n gate:      + x_hi @ q8(16*E)/16, where E = 32W - W8 is the weight
    quantization residual.
All biases for the x-side are folded into the input: xc = x + c with
W8^T c = 32b (min-norm lstsq), so sigmoid/tanh run bias-free and the
r0/r1/z0/z1 pre-activations drain in a single merged 4-bank ACT instruction.
The hidden-side n bias rides the Pool-engine scalar_tensor_tensor.

Blend is out = n + z*(h16 - n) in fp16 (DVE 2x mode); output written fp16
and upcast on host.
"""

import sys

if "/opt/trn_rl_repo" not in sys.path:
    sys.path.insert(0, "/opt/trn_rl_repo")

import numpy as np
import ml_dtypes

F8 = ml_dtypes.float8_e4m3

INPUT_DIM = 1024
HIDDEN_DIM = 2048
NUM_BLOCKS = 8
BS = HIDDEN_DIM // NUM_BLOCKS      # 256
G3 = 3 * BS                        # 768
BATCH = 2048
CW = 256                           # compute chunk (psum fp32 half-bank)
NCH = BATCH // CW                  # 8 compute chunks
NG = NCH // 2                      # 4 dma/elementwise groups of 512
KX = INPUT_DIM // 128              # 8 x k-tiles (4 DR pairs)
KIN = 2 * KX + 2                   # xin k-tiles: x_hi(8) + x_lo(8) + h8(2)

_cached = None


def _build():
    import concourse.tile as tile
    import concourse.mybir as mybir
    from concourse import bacc

    f32 = mybir.dt.float32
    f16 = mybir.dt.float16
    f8 = mybir.dt.float8e4
    ALU = mybir.AluOpType
    ACT = mybir.ActivationFunctionType
    DR = mybir.MatmulPerfMode.DoubleRow

    nc = bacc.Bacc("TRN2", target_bir_lowering=False, debug=False, num_devices=8)

    # xin rows: x_hi (1024, k-major) ++ x_lo (1024) ++ h8 (256); cols batch
    xin = nc.dram_tensor("xin", [128 * KIN, BATCH], f8, kind="ExternalInput")
    # wpk rows: W8_ih (1024 k-major) ++ W8_hh (256); cols gates r|z|n
    wpk = nc.dram_tensor("wpk", [INPUT_DIM + BS, G3], f8, kind="ExternalInput")
    # wcor: q8(16*(32W_ih - W8_ih))[:, n]/16 — the n-gate weight-residual
    # term. (The W8/16 tensors for the x_lo passes are derived on-chip.)
    wcor = nc.dram_tensor("wcor", [INPUT_DIM, BS], f8, kind="ExternalInput")
    # q8(32*b_hh_n) packed [1, 256] — injected into P_hn via a ones-matmul
    bim = nc.dram_tensor("bim", [1, 256], f8, kind="ExternalInput")
    h16 = nc.dram_tensor("h16", [BS, BATCH], f16, kind="ExternalInput")
    oT = nc.dram_tensor("oT", [BS, BATCH], f16, kind="ExternalOutput")

    with tile.TileContext(nc) as tc:
        with (
            tc.tile_pool(name="const", bufs=1) as cp,
            tc.tile_pool(name="xin_p", bufs=3) as xp,
            tc.tile_pool(name="h16_p", bufs=2) as hp,
            tc.tile_pool(name="work", bufs=2) as gp,
            tc.tile_pool(name="psum", bufs=1, space="PSUM") as pp,
        ):
            # PE warm-up: keep the PE continuously busy through the DMA
            # prologue so the p-state ramp (cold -> 2.4GHz after 3us) is done
            # before real matmuls issue.
            wu = cp.tile([128, 32], f8, tag="wu")
            nc.vector.memset(wu[:], 0.0)
            pdummy = pp.tile([128, 2048], f32, tag="rz", name="pdummy")
            for i in range(110):
                nc.tensor.matmul(pdummy[0:32, 0:32], wu[:, 0:32], wu[:],
                                 start=True, stop=True)

            # --- DMA prologue (sync/SP queue, serial in program order).
            # Finely split at the head so chunk-0 matmuls start as early as
            # possible; the emission of compute below is kp-outer to match
            # this arrival order.
            wA = cp.tile([128, 4 * G3], f8, tag="wA")     # W8_ih k0..k3
            wAv = wA[:].rearrange("p (k g) -> p k g", k=4)
            g0 = xp.tile([128, KIN * 512], f8, tag="xg", name="xg0")
            g0v = g0[:].rearrange("p (k b) -> p k b", k=KIN)
            nc.sync.dma_start(
                wAv[:, 0:2, :],
                wpk.ap()[0:256, :].rearrange("(k p) g -> p k g", p=128))
            # bias row for the P_hn injection (partition 0 holds 32*bhn;
            # cols 256:512 stay zero and serve as the second DR k-tile)
            bim_sb = cp.tile([128, 512], f8, tag="bim")
            nc.vector.memset(bim_sb[:], 0.0)
            nc.sync.dma_start(bim_sb[0:1, 0:256], bim.ap())
            ones8 = cp.tile([128, 1024], f8, tag="ones8")
            nc.vector.memset(ones8[:], 1.0)
            nc.sync.dma_start(
                g0v[:, 0:2, :],
                xin.ap()[0:256, 0:512].rearrange("(k p) b -> p k b", p=128))
            nc.sync.dma_start(
                wAv[:, 2:4, :],
                wpk.ap()[256:512, :].rearrange("(k p) g -> p k g", p=128))
            nc.sync.dma_start(
                g0v[:, 2:KX, :],
                xin.ap()[256:1024, 0:512].rearrange("(k p) b -> p k b", p=128))
            wB = cp.tile([128, 6 * G3], f8, tag="wB")     # k4..k7 + hh k0..k1
            wBv0 = wB[:].rearrange("p (k g) -> p k g", k=6)
            nc.sync.dma_start(
                wBv0[:, 0:2, :],
                wpk.ap()[512:768, :].rearrange("(k p) g -> p k g", p=128))
            nc.sync.dma_start(
                wBv0[:, 2:6, :],
                wpk.ap()[768:1280, :].rearrange("(k p) g -> p k g", p=128))
            # h8 ahead of x_lo: the h-side matmuls run before the x_lo passes
            nc.sync.dma_start(
                g0v[:, 2 * KX:KIN, :],
                xin.ap()[2048:128 * KIN, 0:512]
                    .rearrange("(k p) b -> p k b", p=128))
            nc.sync.dma_start(
                g0v[:, KX:2 * KX, :],
                xin.ap()[1024:2048, 0:512]
                    .rearrange("(k p) b -> p k b", p=128))
            # n-gate weight-residual correction weights
            we = cp.tile([128, KX * BS], f8, tag="we")
            nc.sync.dma_start(
                we[:].rearrange("p (k g) -> p k g", k=KX),
                wcor.ap().rearrange("(k p) g -> p k g", p=128))
            h0 = hp.tile([128, 2 * 512], f16, tag="hg", name="hg0")
            nc.sync.dma_start(
                h0[:].rearrange("p (t b) -> p t b", t=2),
                h16.ap()[:, 0:512].rearrange("(t p) b -> p t b", p=128))

            # W8/16 for the x_lo correction passes, derived on-chip on the
            # (head-idle) DVE instead of spending DMA stream bytes; z columns
            # first since the z x_lo passes run before the i_n ones
            wl = cp.tile([128, KX * 2 * BS], f8, tag="wl")
            wBv = wBv0
            wlvd = wl[:].rearrange("p (k g) -> p k g", k=KX)
            nc.vector.tensor_scalar_mul(
                wlvd[:, 0:4, 0:BS], wAv[:, :, BS:2 * BS], 1.0 / 16)
            nc.vector.tensor_scalar_mul(
                wlvd[:, 4:KX, 0:BS], wBv[:, 0:4, BS:2 * BS], 1.0 / 16)
            nc.gpsimd.tensor_scalar_mul(
                wlvd[:, 0:4, BS:2 * BS], wAv[:, :, 2 * BS:G3], 1.0 / 16)
            nc.gpsimd.tensor_scalar_mul(
                wlvd[:, 4:KX, BS:2 * BS], wBv[:, 0:4, 2 * BS:G3], 1.0 / 16)

            # remaining group loads (x_hi part first), still on the SP queue
            # ahead of all output DMAs (SP SEQ is in-order; outputs must not
            # gate inputs)
            xgs, hgs = [None], [h0]
            for g in range(1, NG):
                xg = xp.tile([128, KIN * 512], f8, tag="xg", name=f"xg{g}")
                xgv_ = xg[:].rearrange("p (k b) -> p k b", k=KIN)
                nc.sync.dma_start(
                    xgv_[:, 0:KX, :],
                    xin.ap()[0:1024, g * 512:(g + 1) * 512]
                        .rearrange("(k p) b -> p k b", p=128))
                nc.sync.dma_start(
                    xgv_[:, KX:KIN, :],
                    xin.ap()[1024:128 * KIN, g * 512:(g + 1) * 512]
                        .rearrange("(k p) b -> p k b", p=128))
                xgs.append(xg)
                hg = hp.tile([128, 2 * 512], f16, tag="hg", name=f"hg{g}")
                nc.sync.dma_start(
                    hg[:].rearrange("p (t b) -> p t b", t=2),
                    h16.ap()[:, g * 512:(g + 1) * 512]
                        .rearrange("(t p) b -> p t b", p=128))
                hgs.append(hg)

            # weight views: [128, ktile, gate] f8
            wlv = wlvd
            wev = we[:].rearrange("p (k g) -> p k g", k=KX)

            def w_hi(kp, gs):
                # stationary [128, 2, 128] for x_hi k-pair kp, gate cols gs
                if kp < 2:
                    return wAv[:, 2 * kp:2 * kp + 2, gs]
                return wBv[:, 2 * kp - 4:2 * kp - 2, gs]

            def gsl(t):
                return slice(t * 128, (t + 1) * 128)

            # Chunks: [512, 512, 512, 256, 256] batch columns. Every psum
            # accumulator occupies its own 2KB bank (HW zeroes the whole
            # zero-region when an accumulation group starts), so wider chunks
            # mean fewer chunk-boundary serialization points. The short final
            # chunks keep the post-PE drain chain off the critical path.
            CHUNKS = [(0, 0, 512), (1, 0, 512), (2, 0, 512),
                      (3, 0, 256), (3, 256, 256)]

            def mm_rz_xhi(rz, xgv, cs, w, tiles=range(4), kp_outer=True):
                # kp-outer matches the k-major DMA arrival order
                order = ([(kp, t) for kp in range(4) for t in tiles]
                         if kp_outer else
                         [(kp, t) for t in tiles for kp in range(4)])
                for kp, t in order:
                    nc.tensor.matmul(
                        rz[:, t * 512:t * 512 + w], w_hi(kp, gsl(t)),
                        xgv[:, 2 * kp:2 * kp + 2, cs],
                        start=(kp == 0), stop=False, perf_mode=DR)

            def mm_in_xhi(nh, xgv, cs, w):
                for kp in range(4):
                    for t in range(2):
                        nc.tensor.matmul(
                            nh[:, t * 512:t * 512 + w], w_hi(kp, gsl(4 + t)),
                            xgv[:, 2 * kp:2 * kp + 2, cs],
                            start=(kp == 0), stop=False, perf_mode=DR)

            def mm_hside_rz(rz, xgv, cs, w, tiles, stop=True):
                for t in tiles:                 # h-side closes each rz tile
                    nc.tensor.matmul(
                        rz[:, t * 512:t * 512 + w], wBv[:, 4:6, gsl(t)],
                        xgv[:, 2 * KX:2 * KX + 2, cs],
                        start=False, stop=stop, perf_mode=DR)

            bimv = bim_sb[:].rearrange("p (i u) -> p i u", u=128)
            onesv = ones8[:].rearrange("p (k n) -> p k n", k=2)

            def mm_hn(nh, xgv, cs, w):
                for t in range(2):              # bias inject + h_n
                    nc.tensor.matmul(
                        nh[:, (2 + t) * 512:(2 + t) * 512 + w],
                        bimv[:, t:t + 3:2, :], onesv[:, :, 0:w],
                        start=True, stop=False, perf_mode=DR)
                    nc.tensor.matmul(
                        nh[:, (2 + t) * 512:(2 + t) * 512 + w],
                        wBv[:, 4:6, gsl(4 + t)],
                        xgv[:, 2 * KX:2 * KX + 2, cs],
                        start=False, stop=True, perf_mode=DR)

            def mm_zlo(rz, xgv, cs, w, stop=True):
                for t in range(2):              # z x_lo fix
                    for kp in range(4):
                        nc.tensor.matmul(
                            rz[:, (2 + t) * 512:(2 + t) * 512 + w],
                            wlv[:, 2 * kp:2 * kp + 2, gsl(t)],
                            xgv[:, KX + 2 * kp:KX + 2 * kp + 2, cs],
                            start=False, stop=stop and (kp == 3), perf_mode=DR)

            def mm_inlo(nh, xgv, cs, w):
                for t in range(2):              # i_n x_lo + W-residual fixes
                    for kp in range(4):
                        nc.tensor.matmul(
                            nh[:, t * 512:t * 512 + w],
                            wlv[:, 2 * kp:2 * kp + 2, gsl(2 + t)],
                            xgv[:, KX + 2 * kp:KX + 2 * kp + 2, cs],
                            start=False, stop=False, perf_mode=DR)
                    for kp in range(4):
                        nc.tensor.matmul(
                            nh[:, t * 512:t * 512 + w],
                            wev[:, 2 * kp:2 * kp + 2, gsl(t)],
                            xgv[:, 2 * kp:2 * kp + 2, cs],
                            start=False, stop=(kp == 3), perf_mode=DR)

            def sigmoid_part(rz, rzc, lo, hi, w):
                nc.scalar.activation(
                    rzc[:].rearrange("p (s c) -> p s c", c=512)[:, lo:hi, 0:w],
                    rz[:].rearrange("p (s c) -> p s c", c=512)[:, lo:hi, 0:w],
                    ACT.Sigmoid, scale=1.0 / 32)

            # Software-pipelined post-PE stream: chunk c's sigmoid/a/b2 are
            # emitted with chunk c, but its tanh + blend + output DMA are
            # emitted during chunk c+1, keeping the in-order ACT and DVE
            # queues from stalling on each other's round-trips.
            pending = []   # [(rzc, b2, hg, cstart, goff, w, par)]

            def emit_ab2(nh, rzc, par, ci, w):
                # a = P_hn * r (bias pre-injected; DVE — GPSIMD can't do PSUM)
                at = gp.tile([128, 1024], f32, tag=f"a{par}", name=f"a{ci}")
                nhv = nh[:].rearrange("p (s c) -> p s c", c=512)
                atv = at[:].rearrange("p (t c) -> p t c", t=2)
                rzcv = rzc[:].rearrange("p (s c) -> p s c", c=512)
                nc.vector.tensor_mul(atv[:, :, 0:w], nhv[:, 2:4, 0:w],
                                     rzcv[:, 0:2, 0:w])
                b2 = gp.tile([128, 1024], f32, tag=f"b2{par}", name=f"b2{ci}")
                nc.vector.tensor_add(
                    b2[:].rearrange("p (t c) -> p t c", t=2)[:, :, 0:w],
                    atv[:, :, 0:w], nhv[:, 0:2, 0:w])
                return b2

            def flush_pending(last=False):
                if not pending:
                    return
                rzc, b2, hg_, cstart, goff, w, par, ci = pending.pop()
                gsb = slice(goff, goff + w)
                n_t = gp.tile([128, 1024], f16, tag=f"n{par}", name=f"n{ci}")
                ntv = n_t[:].rearrange("p (t c) -> p t c", t=2)
                nc.scalar.activation(
                    ntv[:, :, 0:w],
                    b2[:].rearrange("p (t c) -> p t c", t=2)[:, :, 0:w],
                    ACT.Tanh, scale=1.0 / 32)
                d_t = gp.tile([128, 1024], f16, tag=f"d{par}", name=f"d{ci}")
                dtv = d_t[:].rearrange("p (t c) -> p t c", t=2)
                # steady-state d runs on the otherwise-idle Pool engine
                # (SBUF-only there); the final chunk keeps DVE for latency
                eng = nc.vector
                eng.tensor_sub(
                    dtv[:, :, 0:w],
                    hg_[:].rearrange("p (t b) -> p t b", t=2)[:, :, gsb],
                    ntv[:, :, 0:w])
                m_t = gp.tile([128, 1024], f16, tag=f"m{par}", name=f"m{ci}")
                mtv = m_t[:].rearrange("p (t c) -> p t c", t=2)
                nc.vector.tensor_mul(
                    mtv[:, :, 0:w],
                    rzc[:].rearrange("p (s c) -> p s c", c=512)[:, 2:4, 0:w],
                    dtv[:, :, 0:w])
                o_t = gp.tile([128, 1024], f16, tag=f"o{par}", name=f"o{ci}")
                otv = o_t[:].rearrange("p (t c) -> p t c", t=2)
                nc.vector.tensor_add(otv[:, :, 0:w], ntv[:, :, 0:w],
                                     mtv[:, :, 0:w])
                nc.sync.dma_start(
                    oT.ap()[:, cstart:cstart + w]
                        .rearrange("(t p) c -> p t c", p=128),
                    otv[:, :, 0:w])

            def chunk_tail(nh, rzc, hg_, cstart, goff, w, par, ci):
                flush_pending()
                b2 = emit_ab2(nh, rzc, par, ci, w)
                pending.append((rzc, b2, hg_, cstart, goff, w, par, ci))

            for ci, (g, goff, w) in enumerate(CHUNKS):
                if g == 0:
                    xgv, hg = g0v, h0
                else:
                    xgv = xgs[g][:].rearrange("p (k b) -> p k b", k=KIN)
                    hg = hgs[g]
                par = "AB"[ci % 2]
                cs = slice(goff, goff + w)
                cstart = g * 512 + goff
                rz = pp.tile([128, 2048], f32, tag="rz", name=f"rz{ci}")
                nh = pp.tile([128, 2048], f32, tag="nh", name=f"nh{ci}")
                rzc = gp.tile([128, 2048], f16, tag=f"rzc{par}",
                              name=f"rzc{ci}")

                if g == 0:
                    # chunk 0 is DMA-starved: z's x_lo pass (whose data
                    # arrives last) goes at the end, with the i_n x_hi wave
                    # filling the gap
                    mm_rz_xhi(rz, xgv, cs, w, tiles=(2, 3, 0, 1))
                    mm_hside_rz(rz, xgv, cs, w, tiles=(2, 3), stop=False)
                    mm_hside_rz(rz, xgv, cs, w, tiles=(0, 1))
                    sigmoid_part(rz, rzc, 0, 2, w)
                    mm_hn(nh, xgv, cs, w)
                    mm_in_xhi(nh, xgv, cs, w)
                    mm_zlo(rz, xgv, cs, w, stop=True)
                    sigmoid_part(rz, rzc, 2, 4, w)
                    mm_inlo(nh, xgv, cs, w)
                else:
                    mm_rz_xhi(rz, xgv, cs, w)
                    mm_zlo(rz, xgv, cs, w, stop=False)
                    mm_hside_rz(rz, xgv, cs, w, tiles=(2, 3))
                    mm_hside_rz(rz, xgv, cs, w, tiles=(0, 1))
                    mm_hn(nh, xgv, cs, w)
                    sigmoid_part(rz, rzc, 0, 4, w)
                    mm_in_xhi(nh, xgv, cs, w)
                    mm_inlo(nh, xgv, cs, w)
                chunk_tail(nh, rzc, hg, cstart, goff, w, par, ci)

            flush_pending(last=True)

    nc.compile()
    return nc


def _get_nc():
    global _cached
    if _cached is None:
        _cached = _build()
    return _cached


def _q8(v):
    return np.asarray(v, np.float32).astype(F8)


def kernel(input, hidden, W_ih, W_hh, b_ih, b_hh):
    input = np.asarray(input, dtype=np.float32)
    hidden = np.asarray(hidden, dtype=np.float32)
    W_ih = np.asarray(W_ih, dtype=np.float32)
    W_hh = np.asarray(W_hh, dtype=np.float32)
    b_ih = np.asarray(b_ih, dtype=np.float32)
    b_hh = np.asarray(b_hh, dtype=np.float32)

    nc = _get_nc()
    from concourse.bass_utils import run_bass_kernel_spmd

    in_maps = []
    for n in range(NUM_BLOCKS):
        WiT = 32.0 * W_ih[n].T                    # [1024, 768] fp32
        WhT = 32.0 * W_hh[n].T                    # [256, 768]
        Wi8 = _q8(WiT)
        Wh8 = _q8(WhT)
        Wi8f = Wi8.astype(np.float32)
        wpk = np.concatenate([Wi8, Wh8], axis=0)  # fp8 [1280, 768]
        E = _q8(16.0 * (WiT - Wi8f)).astype(np.float32)
        wcor = (E[:, 2 * BS:] / 16.0).astype(F8)  # [1024, 256]

        brz = b_ih[n, :2 * BS] + b_hh[n, :2 * BS]
        bin_ = b_ih[n, 2 * BS:]
        bhn = b_hh[n, 2 * BS:]
        b_vec = np.concatenate([brz, bin_]) * 32.0
        c, *_ = np.linalg.lstsq(Wi8f.T, b_vec, rcond=None)

        xc = input + c[None, :]
        x_hi = _q8(xc)
        x_lo = _q8(16.0 * (xc - x_hi.astype(np.float32)))
        hb = hidden[:, n * BS:(n + 1) * BS]
        h8 = _q8(hb)
        xin = np.concatenate(
            [np.ascontiguousarray(x_hi.T), np.ascontiguousarray(x_lo.T),
             np.ascontiguousarray(h8.T)], axis=0)  # [2304, 2048] fp8

        bim_t = np.ascontiguousarray(
            (32.0 * bhn).reshape(1, 256)).astype(F8)

        in_maps.append({
            "xin": xin,
            "wpk": np.ascontiguousarray(wpk),
            "wcor": np.ascontiguousarray(wcor),
            "bim": bim_t,
            "h16": np.ascontiguousarray(hb.T.astype(np.float16)),
        })

    res = run_bass_kernel_spmd(nc, in_maps, core_ids=list(range(NUM_BLOCKS)))
    out = np.empty((BATCH, HIDDEN_DIM), dtype=np.float32)
    for n in range(NUM_BLOCKS):
        out[:, n * BS:(n + 1) * BS] = res.results[n]["oT"].T.astype(np.float32)
    return out


# revision 84
# speedup vs baseline: 1.5027x; 1.0040x over previous
"""BlockGRU Trainium2 kernel — fp8 DoubleRow edition.

Block-diagonal GRU cell: 8 independent blocks (block_size 256), batch 2048,
input_dim 1024. Sharded one block per NeuronCore.

Core idea: all matmuls run as fp8(e4m3) DoubleRow — 2 contraction k-tiles
(256 dims) per instruction at 0.5 cycles/row, 4x the fp16 PE throughput.
Raw e4m3 quantization noise fails the 2e-2 gate (rel-L2 2.6e-2), so the
noise-dominant paths get cheap fp8 correction passes (validated by host-side
simulation, rel-L2 1.3e-2):
  - z & n gates: + x_lo @ (W8/16), where x_lo = q8(16*(xc - x_hi)) recovers
    the input quantization residual (scaled into e4m3's normal range).
  - n gate:      + x_hi @ q8(16*E)/16, where E = 32W - W8 is the weight
    quantization residual.
All biases for the x-side are folded into the input: xc = x + c with
W8^T c = 32b (min-norm lstsq), so sigmoid/tanh run bias-free and the
r0/r1/z0/z1 pre-activations drain in a single merged 4-bank ACT instruction.
The hidden-side n bias rides the Pool-engine scalar_tensor_tensor.

Blend is out = n + z*(h16 - n) in fp16 (DVE 2x mode); output written fp16
and upcast on host.
"""

import sys

if "/opt/trn_rl_repo" not in sys.path:
    sys.path.insert(0, "/opt/trn_rl_repo")

import numpy as np
import ml_dtypes

F8 = ml_dtypes.float8_e4m3

INPUT_DIM = 1024
HIDDEN_DIM = 2048
NUM_BLOCKS = 8
BS = HIDDEN_DIM // NUM_BLOCKS      # 256
G3 = 3 * BS                        # 768
BATCH = 2048
CW = 256                           # compute chunk (psum fp32 half-bank)
NCH = BATCH // CW                  # 8 compute chunks
NG = NCH // 2                      # 4 dma/elementwise groups of 512
KX = INPUT_DIM // 128              # 8 x k-tiles (4 DR pairs)
KIN = 2 * KX + 2                   # xin k-tiles: x_hi(8) + x_lo(8) + h8(2)

_cached = None


def _build():
    import concourse.tile as tile
    import concourse.mybir as mybir
    from concourse import bacc

    f32 = mybir.dt.float32
    f16 = mybir.dt.float16
    f8 = mybir.dt.float8e4
    ALU = mybir.AluOpType
    ACT = mybir.ActivationFunctionType
    DR = mybir.MatmulPerfMode.DoubleRow

    nc = bacc.Bacc("TRN2", target_bir_lowering=False, debug=False, num_devices=8)

    # xin rows: x_hi (1024, k-major) ++ x_lo (1024) ++ h8 (256); cols batch
    xin = nc.dram_tensor("xin", [128 * KIN, BATCH], f8, kind="ExternalInput")
    # wpk rows: W8_ih (1024 k-major) ++ W8_hh (256); cols gates r|z|n
    wpk = nc.dram_tensor("wpk", [INPUT_DIM + BS, G3], f8, kind="ExternalInput")
    # wcor: q8(16*(32W_ih - W8_ih))[:, n]/16 — the n-gate weight-residual
    # term. (The W8/16 tensors for the x_lo passes are derived on-chip.)
    wcor = nc.dram_tensor("wcor", [INPUT_DIM, BS], f8, kind="ExternalInput")
    # q8(32*b_hh_n) packed [1, 256] — injected into P_hn via a ones-matmul
    bim = nc.dram_tensor("bim", [1, 256], f8, kind="ExternalInput")
    h16 = nc.dram_tensor("h16", [BS, BATCH], f16, kind="ExternalInput")
    oT = nc.dram_tensor("oT", [BS, BATCH], f16, kind="ExternalOutput")

    with tile.TileContext(nc) as tc:
        with (
            tc.tile_pool(name="const", bufs=1) as cp,
            tc.tile_pool(name="xin_p", bufs=3) as xp,
            tc.tile_pool(name="h16_p", bufs=2) as hp,
            tc.tile_pool(name="work", bufs=2) as gp,
            tc.tile_pool(name="psum", bufs=1, space="PSUM") as pp,
        ):
            # PE warm-up: keep the PE continuously busy through the DMA
            # prologue so the p-state ramp (cold -> 2.4GHz after 3us) is done
            # before real matmuls issue.
            wu = cp.tile([128, 32], f8, tag="wu")
            nc.vector.memset(wu[:], 0.0)
            pdummy = pp.tile([128, 2048], f32, tag="rz", name="pdummy")
            for i in range(110):
                nc.tensor.matmul(pdummy[0:32, 0:32], wu[:, 0:32], wu[:],
                                 start=True, stop=True)

            # --- DMA prologue (sync/SP queue, serial in program order).
            # Finely split at the head so chunk-0 matmuls start as early as
            # possible; the emission of compute below is kp-outer to match
            # this arrival order.
            wA = cp.tile([128, 4 * G3], f8, tag="wA")     # W8_ih k0..k3
            wAv = wA[:].rearrange("p (k g) -> p k g", k=4)
            g0 = xp.tile([128, KIN * 512], f8, tag="xg", name="xg0")
            g0v = g0[:].rearrange("p (k b) -> p k b", k=KIN)
            nc.sync.dma_start(
                wAv[:, 0:2, :],
                wpk.ap()[0:256, :].rearrange("(k p) g -> p k g", p=128))
            # bias row for the P_hn injection (partition 0 holds 32*bhn;
            # cols 256:512 stay zero and serve as the second DR k-tile)
            bim_sb = cp.tile([128, 512], f8, tag="bim")
            nc.vector.memset(bim_sb[:], 0.0)
            nc.sync.dma_start(bim_sb[0:1, 0:256], bim.ap())
            ones8 = cp.tile([128, 1024], f8, tag="ones8")
            nc.vector.memset(ones8[:], 1.0)
            nc.sync.dma_start(
                g0v[:, 0:2, :],
                xin.ap()[0:256, 0:512].rearrange("(k p) b -> p k b", p=128))
            nc.sync.dma_start(
                wAv[:, 2:4, :],
                wpk.ap()[256:512, :].rearrange("(k p) g -> p k g", p=128))
            nc.sync.dma_start(
                g0v[:, 2:KX, :],
                xin.ap()[256:1024, 0:512].rearrange("(k p) b -> p k b", p=128))
            wB = cp.tile([128, 6 * G3], f8, tag="wB")     # k4..k7 + hh k0..k1
            wBv0 = wB[:].rearrange("p (k g) -> p k g", k=6)
            nc.sync.dma_start(
                wBv0[:, 0:2, :],
                wpk.ap()[512:768, :].rearrange("(k p) g -> p k g", p=128))
            nc.sync.dma_start(
                wBv0[:, 2:6, :],
                wpk.ap()[768:1280, :].rearrange("(k p) g -> p k g", p=128))
            # h8 ahead of x_lo: the h-side matmuls run before the x_lo passes
            nc.sync.dma_start(
                g0v[:, 2 * KX:KIN, :],
                xin.ap()[2048:128 * KIN, 0:512]
                    .rearrange("(k p) b -> p k b", p=128))
            nc.sync.dma_start(
                g0v[:, KX:2 * KX, :],
                xin.ap()[1024:2048, 0:512]
                    .rearrange("(k p) b -> p k b", p=128))
            # n-gate weight-residual correction weights
            we = cp.tile([128, KX * BS], f8, tag="we")
            nc.sync.dma_start(
                we[:].rearrange("p (k g) -> p k g", k=KX),
                wcor.ap().rearrange("(k p) g -> p k g", p=128))
            h0 = hp.tile([128, 2 * 512], f16, tag="hg", name="hg0")
            nc.sync.dma_start(
                h0[:].rearrange("p (t b) -> p t b", t=2),
                h16.ap()[:, 0:512].rearrange("(t p) b -> p t b", p=128))

            # W8/16 for the x_lo correction passes, derived on-chip on the
            # (head-idle) DVE instead of spending DMA stream bytes; z columns
            # first since the z x_lo passes run before the i_n ones
            wl = cp.tile([128, KX * 2 * BS], f8, tag="wl")
            wBv = wBv0
            wlvd = wl[:].rearrange("p (k g) -> p k g", k=KX)
            nc.vector.tensor_scalar_mul(
                wlvd[:, 0:4, 0:BS], wAv[:, :, BS:2 * BS], 1.0 / 16)
            nc.vector.tensor_scalar_mul(
                wlvd[:, 4:KX, 0:BS], wBv[:, 0:4, BS:2 * BS], 1.0 / 16)
            nc.gpsimd.tensor_scalar_mul(
                wlvd[:, 0:4, BS:2 * BS], wAv[:, :, 2 * BS:G3], 1.0 / 16)
            nc.gpsimd.tensor_scalar_mul(
                wlvd[:, 4:KX, BS:2 * BS], wBv[:, 0:4, 2 * BS:G3], 1.0 / 16)

            # remaining group loads (x_hi part first), still on the SP queue
            # ahead of all output DMAs (SP SEQ is in-order; outputs must not
            # gate inputs)
            xgs, hgs = [None], [h0]
            for g in range(1, NG):
                xg = xp.tile([128, KIN * 512], f8, tag="xg", name=f"xg{g}")
                xgv_ = xg[:].rearrange("p (k b) -> p k b", k=KIN)
                nc.sync.dma_start(
                    xgv_[:, 0:KX, :],
                    xin.ap()[0:1024, g * 512:(g + 1) * 512]
                        .rearrange("(k p) b -> p k b", p=128))
                nc.sync.dma_start(
                    xgv_[:, KX:KIN, :],
                    xin.ap()[1024:128 * KIN, g * 512:(g + 1) * 512]
                        .rearrange("(k p) b -> p k b", p=128))
                xgs.append(xg)
                hg = hp.tile([128, 2 * 512], f16, tag="hg", name=f"hg{g}")
                nc.sync.dma_start(
                    hg[:].rearrange("p (t b) -> p t b", t=2),
                    h16.ap()[:, g * 512:(g + 1) * 512]
                        .rearrange("(t p) b -> p t b", p=128))
                hgs.append(hg)

            # weight views: [128, ktile, gate] f8
            wlv = wlvd
            wev = we[:].rearrange("p (k g) -> p k g", k=KX)

            def w_hi(kp, gs):
                # stationary [128, 2, 128] for x_hi k-pair kp, gate cols gs
                if kp < 2:
                    return wAv[:, 2 * kp:2 * kp + 2, gs]
                return wBv[:, 2 * kp - 4:2 * kp - 2, gs]

            def gsl(t):
                return slice(t * 128, (t + 1) * 128)

            # Chunks: [512, 512, 512, 256, 256] batch columns. Every psum
            # accumulator occupies its own 2KB bank (HW zeroes the whole
            # zero-region when an accumulation group starts), so wider chunks
            # mean fewer chunk-boundary serialization points. The short final
            # chunks keep the post-PE drain chain off the critical path.
            CHUNKS = [(0, 0, 512), (1, 0, 512), (2, 0, 512),
                      (3, 0, 384), (3, 384, 128)]

            def mm_rz_xhi(rz, xgv, cs, w, tiles=range(4), kp_outer=True):
                # kp-outer matches the k-major DMA arrival order
                order = ([(kp, t) for kp in range(4) for t in tiles]
                         if kp_outer else
                         [(kp, t) for t in tiles for kp in range(4)])
                for kp, t in order:
                    nc.tensor.matmul(
                        rz[:, t * 512:t * 512 + w], w_hi(kp, gsl(t)),
                        xgv[:, 2 * kp:2 * kp + 2, cs],
                        start=(kp == 0), stop=False, perf_mode=DR)

            def mm_in_xhi(nh, xgv, cs, w):
                for kp in range(4):
                    for t in range(2):
                        nc.tensor.matmul(
                            nh[:, t * 512:t * 512 + w], w_hi(kp, gsl(4 + t)),
                            xgv[:, 2 * kp:2 * kp + 2, cs],
                            start=(kp == 0), stop=False, perf_mode=DR)

            def mm_hside_rz(rz, xgv, cs, w, tiles, stop=True):
                for t in tiles:                 # h-side closes each rz tile
                    nc.tensor.matmul(
                        rz[:, t * 512:t * 512 + w], wBv[:, 4:6, gsl(t)],
                        xgv[:, 2 * KX:2 * KX + 2, cs],
                        start=False, stop=stop, perf_mode=DR)

            bimv = bim_sb[:].rearrange("p (i u) -> p i u", u=128)
            onesv = ones8[:].rearrange("p (k n) -> p k n", k=2)

            def mm_hn(nh, xgv, cs, w):
                for t in range(2):              # bias inject + h_n
                    nc.tensor.matmul(
                        nh[:, (2 + t) * 512:(2 + t) * 512 + w],
                        bimv[:, t:t + 3:2, :], onesv[:, :, 0:w],
                        start=True, stop=False, perf_mode=DR)
                    nc.tensor.matmul(
                        nh[:, (2 + t) * 512:(2 + t) * 512 + w],
                        wBv[:, 4:6, gsl(4 + t)],
                        xgv[:, 2 * KX:2 * KX + 2, cs],
                        start=False, stop=True, perf_mode=DR)

            def mm_zlo(rz, xgv, cs, w, stop=True):
                for t in range(2):              # z x_lo fix
                    for kp in range(4):
                        nc.tensor.matmul(
                            rz[:, (2 + t) * 512:(2 + t) * 512 + w],
                            wlv[:, 2 * kp:2 * kp + 2, gsl(t)],
                            xgv[:, KX + 2 * kp:KX + 2 * kp + 2, cs],
                            start=False, stop=stop and (kp == 3), perf_mode=DR)

            def mm_inlo(nh, xgv, cs, w):
                for t in range(2):              # i_n x_lo + W-residual fixes
                    for kp in range(4):
                        nc.tensor.matmul(
                            nh[:, t * 512:t * 512 + w],
                            wlv[:, 2 * kp:2 * kp + 2, gsl(2 + t)],
                            xgv[:, KX + 2 * kp:KX + 2 * kp + 2, cs],
                            start=False, stop=False, perf_mode=DR)
                    for kp in range(4):
                        nc.tensor.matmul(
                            nh[:, t * 512:t * 512 + w],
                            wev[:, 2 * kp:2 * kp + 2, gsl(t)],
                            xgv[:, 2 * kp:2 * kp + 2, cs],
                            start=False, stop=(kp == 3), perf_mode=DR)

            def sigmoid_part(rz, rzc, lo, hi, w):
                nc.scalar.activation(
                    rzc[:].rearrange("p (s c) -> p s c", c=512)[:, lo:hi, 0:w],
                    rz[:].rearrange("p (s c) -> p s c", c=512)[:, lo:hi, 0:w],
                    ACT.Sigmoid, scale=1.0 / 32)

            # Software-pipelined post-PE stream: chunk c's sigmoid/a/b2 are
            # emitted with chunk c, but its tanh + blend + output DMA are
            # emitted during chunk c+1, keeping the in-order ACT and DVE
            # queues from stalling on each other's round-trips.
            pending = []   # [(rzc, b2, hg, cstart, goff, w, par)]

            def emit_ab2(nh, rzc, par, ci, w):
                # a = P_hn * r (bias pre-injected; DVE — GPSIMD can't do PSUM)
                at = gp.tile([128, 1024], f32, tag=f"a{par}", name=f"a{ci}")
                nhv = nh[:].rearrange("p (s c) -> p s c", c=512)
                atv = at[:].rearrange("p (t c) -> p t c", t=2)
                rzcv = rzc[:].rearrange("p (s c) -> p s c", c=512)
                nc.vector.tensor_mul(atv[:, :, 0:w], nhv[:, 2:4, 0:w],
                                     rzcv[:, 0:2, 0:w])
                b2 = gp.tile([128, 1024], f32, tag=f"b2{par}", name=f"b2{ci}")
                nc.vector.tensor_add(
                    b2[:].rearrange("p (t c) -> p t c", t=2)[:, :, 0:w],
                    atv[:, :, 0:w], nhv[:, 0:2, 0:w])
                return b2

            def flush_pending(last=False):
                if not pending:
                    return
                rzc, b2, hg_, cstart, goff, w, par, ci = pending.pop()
                gsb = slice(goff, goff + w)
                n_t = gp.tile([128, 1024], f16, tag=f"n{par}", name=f"n{ci}")
                ntv = n_t[:].rearrange("p (t c) -> p t c", t=2)
                nc.scalar.activation(
                    ntv[:, :, 0:w],
                    b2[:].rearrange("p (t c) -> p t c", t=2)[:, :, 0:w],
                    ACT.Tanh, scale=1.0 / 32)
                d_t = gp.tile([128, 1024], f16, tag=f"d{par}", name=f"d{ci}")
                dtv = d_t[:].rearrange("p (t c) -> p t c", t=2)
                # steady-state d runs on the otherwise-idle Pool engine
                # (SBUF-only there); the final chunk keeps DVE for latency
                eng = nc.vector
                eng.tensor_sub(
                    dtv[:, :, 0:w],
                    hg_[:].rearrange("p (t b) -> p t b", t=2)[:, :, gsb],
                    ntv[:, :, 0:w])
                m_t = gp.tile([128, 1024], f16, tag=f"m{par}", name=f"m{ci}")
                mtv = m_t[:].rearrange("p (t c) -> p t c", t=2)
                nc.vector.tensor_mul(
                    mtv[:, :, 0:w],
                    rzc[:].rearrange("p (s c) -> p s c", c=512)[:, 2:4, 0:w],
                    dtv[:, :, 0:w])
                o_t = gp.tile([128, 1024], f16, tag=f"o{par}", name=f"o{ci}")
                otv = o_t[:].rearrange("p (t c) -> p t c", t=2)
                nc.vector.tensor_add(otv[:, :, 0:w], ntv[:, :, 0:w],
                                     mtv[:, :, 0:w])
                nc.sync.dma_start(
                    oT.ap()[:, cstart:cstart + w]
                        .rearrange("(t p) c -> p t c", p=128),
                    otv[:, :, 0:w])

            def chunk_tail(nh, rzc, hg_, cstart, goff, w, par, ci):
                flush_pending()
                b2 = emit_ab2(nh, rzc, par, ci, w)
                pending.append((rzc, b2, hg_, cstart, goff, w, par, ci))

            for ci, (g, goff, w) in enumerate(CHUNKS):
                if g == 0:
                    xgv, hg = g0v, h0
                else:
                    xgv = xgs[g][:].rearrange("p (k b) -> p k b", k=KIN)
                    hg = hgs[g]
                par = "AB"[ci % 2]
                cs = slice(goff, goff + w)
                cstart = g * 512 + goff
                rz = pp.tile([128, 2048], f32, tag="rz", name=f"rz{ci}")
                nh = pp.tile([128, 2048], f32, tag="nh", name=f"nh{ci}")
                rzc = gp.tile([128, 2048], f16, tag=f"rzc{par}",
                              name=f"rzc{ci}")

                if g == 0:
                    # chunk 0 is DMA-starved: z's x_lo pass (whose data
                    # arrives last) goes at the end, with the i_n x_hi wave
                    # filling the gap
                    mm_rz_xhi(rz, xgv, cs, w, tiles=(2, 3, 0, 1))
                    mm_hside_rz(rz, xgv, cs, w, tiles=(2, 3), stop=False)
                    mm_hside_rz(rz, xgv, cs, w, tiles=(0, 1))
                    sigmoid_part(rz, rzc, 0, 2, w)
                    mm_hn(nh, xgv, cs, w)
                    mm_in_xhi(nh, xgv, cs, w)
                    mm_zlo(rz, xgv, cs, w, stop=True)
                    sigmoid_part(rz, rzc, 2, 4, w)
                    mm_inlo(nh, xgv, cs, w)
                else:
                    mm_rz_xhi(rz, xgv, cs, w)
                    mm_zlo(rz, xgv, cs, w, stop=False)
                    mm_hside_rz(rz, xgv, cs, w, tiles=(2, 3))
                    mm_hside_rz(rz, xgv, cs, w, tiles=(0, 1))
                    mm_hn(nh, xgv, cs, w)
                    sigmoid_part(rz, rzc, 0, 4, w)
                    mm_in_xhi(nh, xgv, cs, w)
                    mm_inlo(nh, xgv, cs, w)
                chunk_tail(nh, rzc, hg, cstart, goff, w, par, ci)

            flush_pending(last=True)

    nc.compile()
    return nc


def _get_nc():
    global _cached
    if _cached is None:
        _cached = _build()
    return _cached


def _q8(v):
    return np.asarray(v, np.float32).astype(F8)


def kernel(input, hidden, W_ih, W_hh, b_ih, b_hh):
    input = np.asarray(input, dtype=np.float32)
    hidden = np.asarray(hidden, dtype=np.float32)
    W_ih = np.asarray(W_ih, dtype=np.float32)
    W_hh = np.asarray(W_hh, dtype=np.float32)
    b_ih = np.asarray(b_ih, dtype=np.float32)
    b_hh = np.asarray(b_hh, dtype=np.float32)

    nc = _get_nc()
    from concourse.bass_utils import run_bass_kernel_spmd

    in_maps = []
    for n in range(NUM_BLOCKS):
        WiT = 32.0 * W_ih[n].T                    # [1024, 768] fp32
        WhT = 32.0 * W_hh[n].T                    # [256, 768]
        Wi8 = _q8(WiT)
        Wh8 = _q8(WhT)
        Wi8f = Wi8.astype(np.float32)
        wpk = np.concatenate([Wi8, Wh8], axis=0)  # fp8 [1280, 768]
        E = _q8(16.0 * (WiT - Wi8f)).astype(np.float32)
        wcor = (E[:, 2 * BS:] / 16.0).astype(F8)  # [1024, 256]

        brz = b_ih[n, :2 * BS] + b_hh[n, :2 * BS]
        bin_ = b_ih[n, 2 * BS:]
        bhn = b_hh[n, 2 * BS:]
        b_vec = np.concatenate([brz, bin_]) * 32.0
        c, *_ = np.linalg.lstsq(Wi8f.T, b_vec, rcond=None)

        xc = input + c[None, :]
        x_hi = _q8(xc)
        x_lo = _q8(16.0 * (xc - x_hi.astype(np.float32)))
        hb = hidden[:, n * BS:(n + 1) * BS]
        h8 = _q8(hb)
        xin = np.concatenate(
            [np.ascontiguousarray(x_hi.T), np.ascontiguousarray(x_lo.T),
             np.ascontiguousarray(h8.T)], axis=0)  # [2304, 2048] fp8

        bim_t = np.ascontiguousarray(
            (32.0 * bhn).reshape(1, 256)).astype(F8)

        in_maps.append({
            "xin": xin,
            "wpk": np.ascontiguousarray(wpk),
            "wcor": np.ascontiguousarray(wcor),
            "bim": bim_t,
            "h16": np.ascontiguousarray(hb.T.astype(np.float16)),
        })

    res = run_bass_kernel_spmd(nc, in_maps, core_ids=list(range(NUM_BLOCKS)))
    out = np.empty((BATCH, HIDDEN_DIM), dtype=np.float32)
    for n in range(NUM_BLOCKS):
        out[:, n * BS:(n + 1) * BS] = res.results[n]["oT"].T.astype(np.float32)
    return out


# revision 87
# speedup vs baseline: 1.5039x; 1.0008x over previous
"""BlockGRU Trainium2 kernel — fp8 DoubleRow edition.

Block-diagonal GRU cell: 8 independent blocks (block_size 256), batch 2048,
input_dim 1024. Sharded one block per NeuronCore.

Core idea: all matmuls run as fp8(e4m3) DoubleRow — 2 contraction k-tiles
(256 dims) per instruction at 0.5 cycles/row, 4x the fp16 PE throughput.
Raw e4m3 quantization noise fails the 2e-2 gate (rel-L2 2.6e-2), so the
noise-dominant paths get cheap fp8 correction passes (validated by host-side
simulation, rel-L2 1.3e-2):
  - z & n gates: + x_lo @ (W8/16), where x_lo = q8(16*(xc - x_hi)) recovers
    the input quantization residual (scaled into e4m3's normal range).
  - n gate:      + x_hi @ q8(16*E)/16, where E = 32W - W8 is the weight
    quantization residual.
All biases for the x-side are folded into the input: xc = x + c with
W8^T c = 32b (min-norm lstsq), so sigmoid/tanh run bias-free and the
r0/r1/z0/z1 pre-activations drain in a single merged 4-bank ACT instruction.
The hidden-side n bias rides the Pool-engine scalar_tensor_tensor.

Blend is out = n + z*(h16 - n) in fp16 (DVE 2x mode); output written fp16
and upcast on host.
"""

import sys

if "/opt/trn_rl_repo" not in sys.path:
    sys.path.insert(0, "/opt/trn_rl_repo")

import numpy as np
import ml_dtypes

F8 = ml_dtypes.float8_e4m3

INPUT_DIM = 1024
HIDDEN_DIM = 2048
NUM_BLOCKS = 8
BS = HIDDEN_DIM // NUM_BLOCKS      # 256
G3 = 3 * BS                        # 768
BATCH = 2048
CW = 256                           # compute chunk (psum fp32 half-bank)
NCH = BATCH // CW                  # 8 compute chunks
NG = NCH // 2                      # 4 dma/elementwise groups of 512
KX = INPUT_DIM // 128              # 8 x k-tiles (4 DR pairs)
KIN = 2 * KX + 2                   # xin k-tiles: x_hi(8) + x_lo(8) + h8(2)

_cached = None


def _build():
    import concourse.tile as tile
    import concourse.mybir as mybir
    from concourse import bacc

    f32 = mybir.dt.float32
    f16 = mybir.dt.float16
    f8 = mybir.dt.float8e4
    ALU = mybir.AluOpType
    ACT = mybir.ActivationFunctionType
    DR = mybir.MatmulPerfMode.DoubleRow

    nc = bacc.Bacc("TRN2", target_bir_lowering=False, debug=False, num_devices=8)

    # xin rows: x_hi (1024, k-major) ++ x_lo (1024) ++ h8 (256); cols batch
    xin = nc.dram_tensor("xin", [128 * KIN, BATCH], f8, kind="ExternalInput")
    # wpk rows: W8_ih (1024 k-major) ++ W8_hh (256); cols gates r|z|n
    wpk = nc.dram_tensor("wpk", [INPUT_DIM + BS, G3], f8, kind="ExternalInput")
    # wcor: q8(16*(32W_ih - W8_ih))[:, n]/16 — the n-gate weight-residual
    # term. (The W8/16 tensors for the x_lo passes are derived on-chip.)
    wcor = nc.dram_tensor("wcor", [INPUT_DIM, BS], f8, kind="ExternalInput")
    # q8(32*b_hh_n) packed [1, 256] — injected into P_hn via a ones-matmul
    bim = nc.dram_tensor("bim", [1, 256], f8, kind="ExternalInput")
    h16 = nc.dram_tensor("h16", [BS, BATCH], f16, kind="ExternalInput")
    oT = nc.dram_tensor("oT", [BS, BATCH], f16, kind="ExternalOutput")

    with tile.TileContext(nc) as tc:
        with (
            tc.tile_pool(name="const", bufs=1) as cp,
            tc.tile_pool(name="xin_p", bufs=3) as xp,
            tc.tile_pool(name="h16_p", bufs=2) as hp,
            tc.tile_pool(name="work", bufs=2) as gp,
            tc.tile_pool(name="psum", bufs=1, space="PSUM") as pp,
        ):
            # PE warm-up: keep the PE continuously busy through the DMA
            # prologue so the p-state ramp (cold -> 2.4GHz after 3us) is done
            # before real matmuls issue.
            wu = cp.tile([128, 32], f8, tag="wu")
            nc.vector.memset(wu[:], 0.0)
            pdummy = pp.tile([128, 2048], f32, tag="rz", name="pdummy")
            for i in range(110):
                nc.tensor.matmul(pdummy[0:32, 0:32], wu[:, 0:32], wu[:],
                                 start=True, stop=True)

            # --- DMA prologue (sync/SP queue, serial in program order).
            # Finely split at the head so chunk-0 matmuls start as early as
            # possible; the emission of compute below is kp-outer to match
            # this arrival order.
            wA = cp.tile([128, 4 * G3], f8, tag="wA")     # W8_ih k0..k3
            wAv = wA[:].rearrange("p (k g) -> p k g", k=4)
            g0 = xp.tile([128, KIN * 512], f8, tag="xg", name="xg0")
            g0v = g0[:].rearrange("p (k b) -> p k b", k=KIN)
            nc.sync.dma_start(
                wAv[:, 0:2, :],
                wpk.ap()[0:256, :].rearrange("(k p) g -> p k g", p=128))
            # bias row for the P_hn injection (partition 0 holds 32*bhn;
            # cols 256:512 stay zero and serve as the second DR k-tile)
            bim_sb = cp.tile([128, 512], f8, tag="bim")
            nc.vector.memset(bim_sb[:], 0.0)
            nc.sync.dma_start(bim_sb[0:1, 0:256], bim.ap())
            ones8 = cp.tile([128, 1024], f8, tag="ones8")
            nc.vector.memset(ones8[:], 1.0)
            nc.sync.dma_start(
                g0v[:, 0:2, :],
                xin.ap()[0:256, 0:512].rearrange("(k p) b -> p k b", p=128))
            nc.sync.dma_start(
                wAv[:, 2:4, :],
                wpk.ap()[256:512, :].rearrange("(k p) g -> p k g", p=128))
            nc.sync.dma_start(
                g0v[:, 2:KX, :],
                xin.ap()[256:1024, 0:512].rearrange("(k p) b -> p k b", p=128))
            wB = cp.tile([128, 6 * G3], f8, tag="wB")     # k4..k7 + hh k0..k1
            wBv0 = wB[:].rearrange("p (k g) -> p k g", k=6)
            nc.sync.dma_start(
                wBv0[:, 0:2, :],
                wpk.ap()[512:768, :].rearrange("(k p) g -> p k g", p=128))
            nc.sync.dma_start(
                wBv0[:, 2:6, :],
                wpk.ap()[768:1280, :].rearrange("(k p) g -> p k g", p=128))
            # h8 ahead of x_lo: the h-side matmuls run before the x_lo passes
            nc.sync.dma_start(
                g0v[:, 2 * KX:KIN, :],
                xin.ap()[2048:128 * KIN, 0:512]
                    .rearrange("(k p) b -> p k b", p=128))
            nc.sync.dma_start(
                g0v[:, KX:2 * KX, :],
                xin.ap()[1024:2048, 0:512]
                    .rearrange("(k p) b -> p k b", p=128))
            # n-gate weight-residual correction weights
            we = cp.tile([128, KX * BS], f8, tag="we")
            nc.sync.dma_start(
                we[:].rearrange("p (k g) -> p k g", k=KX),
                wcor.ap().rearrange("(k p) g -> p k g", p=128))
            h0 = hp.tile([128, 2 * 512], f16, tag="hg", name="hg0")
            nc.sync.dma_start(
                h0[:].rearrange("p (t b) -> p t b", t=2),
                h16.ap()[:, 0:512].rearrange("(t p) b -> p t b", p=128))

            # W8/16 for the x_lo correction passes, derived on-chip on the
            # (head-idle) DVE instead of spending DMA stream bytes; z columns
            # first since the z x_lo passes run before the i_n ones
            wl = cp.tile([128, KX * 2 * BS], f8, tag="wl")
            wBv = wBv0
            wlvd = wl[:].rearrange("p (k g) -> p k g", k=KX)
            nc.vector.tensor_scalar_mul(
                wlvd[:, 0:4, 0:BS], wAv[:, :, BS:2 * BS], 1.0 / 16)
            nc.vector.tensor_scalar_mul(
                wlvd[:, 4:KX, 0:BS], wBv[:, 0:4, BS:2 * BS], 1.0 / 16)
            nc.gpsimd.tensor_scalar_mul(
                wlvd[:, 0:4, BS:2 * BS], wAv[:, :, 2 * BS:G3], 1.0 / 16)
            nc.gpsimd.tensor_scalar_mul(
                wlvd[:, 4:KX, BS:2 * BS], wBv[:, 0:4, 2 * BS:G3], 1.0 / 16)

            # remaining group loads (x_hi part first), still on the SP queue
            # ahead of all output DMAs (SP SEQ is in-order; outputs must not
            # gate inputs)
            xgs, hgs = [None], [h0]
            for g in range(1, NG):
                xg = xp.tile([128, KIN * 512], f8, tag="xg", name=f"xg{g}")
                xgv_ = xg[:].rearrange("p (k b) -> p k b", k=KIN)
                nc.sync.dma_start(
                    xgv_[:, 0:KX, :],
                    xin.ap()[0:1024, g * 512:(g + 1) * 512]
                        .rearrange("(k p) b -> p k b", p=128))
                nc.sync.dma_start(
                    xgv_[:, KX:KIN, :],
                    xin.ap()[1024:128 * KIN, g * 512:(g + 1) * 512]
                        .rearrange("(k p) b -> p k b", p=128))
                xgs.append(xg)
                hg = hp.tile([128, 2 * 512], f16, tag="hg", name=f"hg{g}")
                nc.sync.dma_start(
                    hg[:].rearrange("p (t b) -> p t b", t=2),
                    h16.ap()[:, g * 512:(g + 1) * 512]
                        .rearrange("(t p) b -> p t b", p=128))
                hgs.append(hg)

            # weight views: [128, ktile, gate] f8
            wlv = wlvd
            wev = we[:].rearrange("p (k g) -> p k g", k=KX)

            def w_hi(kp, gs):
                # stationary [128, 2, 128] for x_hi k-pair kp, gate cols gs
                if kp < 2:
                    return wAv[:, 2 * kp:2 * kp + 2, gs]
                return wBv[:, 2 * kp - 4:2 * kp - 2, gs]

            def gsl(t):
                return slice(t * 128, (t + 1) * 128)

            # Chunks: [512, 512, 512, 256, 256] batch columns. Every psum
            # accumulator occupies its own 2KB bank (HW zeroes the whole
            # zero-region when an accumulation group starts), so wider chunks
            # mean fewer chunk-boundary serialization points. The short final
            # chunks keep the post-PE drain chain off the critical path.
            CHUNKS = [(0, 0, 512), (1, 0, 512), (2, 0, 512),
                      (3, 0, 320), (3, 320, 192)]

            def mm_rz_xhi(rz, xgv, cs, w, tiles=range(4), kp_outer=True):
                # kp-outer matches the k-major DMA arrival order
                order = ([(kp, t) for kp in range(4) for t in tiles]
                         if kp_outer else
                         [(kp, t) for t in tiles for kp in range(4)])
                for kp, t in order:
                    nc.tensor.matmul(
                        rz[:, t * 512:t * 512 + w], w_hi(kp, gsl(t)),
                        xgv[:, 2 * kp:2 * kp + 2, cs],
                        start=(kp == 0), stop=False, perf_mode=DR)

            def mm_in_xhi(nh, xgv, cs, w):
                for kp in range(4):
                    for t in range(2):
                        nc.tensor.matmul(
                            nh[:, t * 512:t * 512 + w], w_hi(kp, gsl(4 + t)),
                            xgv[:, 2 * kp:2 * kp + 2, cs],
                            start=(kp == 0), stop=False, perf_mode=DR)

            def mm_hside_rz(rz, xgv, cs, w, tiles, stop=True):
                for t in tiles:                 # h-side closes each rz tile
                    nc.tensor.matmul(
                        rz[:, t * 512:t * 512 + w], wBv[:, 4:6, gsl(t)],
                        xgv[:, 2 * KX:2 * KX + 2, cs],
                        start=False, stop=stop, perf_mode=DR)

            bimv = bim_sb[:].rearrange("p (i u) -> p i u", u=128)
            onesv = ones8[:].rearrange("p (k n) -> p k n", k=2)

            def mm_hn(nh, xgv, cs, w):
                for t in range(2):              # bias inject + h_n
                    nc.tensor.matmul(
                        nh[:, (2 + t) * 512:(2 + t) * 512 + w],
                        bimv[:, t:t + 3:2, :], onesv[:, :, 0:w],
                        start=True, stop=False, perf_mode=DR)
                    nc.tensor.matmul(
                        nh[:, (2 + t) * 512:(2 + t) * 512 + w],
                        wBv[:, 4:6, gsl(4 + t)],
                        xgv[:, 2 * KX:2 * KX + 2, cs],
                        start=False, stop=True, perf_mode=DR)

            def mm_zlo(rz, xgv, cs, w, stop=True):
                for t in range(2):              # z x_lo fix
                    for kp in range(4):
                        nc.tensor.matmul(
                            rz[:, (2 + t) * 512:(2 + t) * 512 + w],
                            wlv[:, 2 * kp:2 * kp + 2, gsl(t)],
                            xgv[:, KX + 2 * kp:KX + 2 * kp + 2, cs],
                            start=False, stop=stop and (kp == 3), perf_mode=DR)

            def mm_inlo(nh, xgv, cs, w):
                for t in range(2):              # i_n x_lo + W-residual fixes
                    for kp in range(4):
                        nc.tensor.matmul(
                            nh[:, t * 512:t * 512 + w],
                            wlv[:, 2 * kp:2 * kp + 2, gsl(2 + t)],
                            xgv[:, KX + 2 * kp:KX + 2 * kp + 2, cs],
                            start=False, stop=False, perf_mode=DR)
                    for kp in range(4):
                        nc.tensor.matmul(
                            nh[:, t * 512:t * 512 + w],
                            wev[:, 2 * kp:2 * kp + 2, gsl(t)],
                            xgv[:, 2 * kp:2 * kp + 2, cs],
                            start=False, stop=(kp == 3), perf_mode=DR)

            def sigmoid_part(rz, rzc, lo, hi, w):
                nc.scalar.activation(
                    rzc[:].rearrange("p (s c) -> p s c", c=512)[:, lo:hi, 0:w],
                    rz[:].rearrange("p (s c) -> p s c", c=512)[:, lo:hi, 0:w],
                    ACT.Sigmoid, scale=1.0 / 32)

            # Software-pipelined post-PE stream: chunk c's sigmoid/a/b2 are
            # emitted with chunk c, but its tanh + blend + output DMA are
            # emitted during chunk c+1, keeping the in-order ACT and DVE
            # queues from stalling on each other's round-trips.
            pending = []   # [(rzc, b2, hg, cstart, goff, w, par)]

            def emit_ab2(nh, rzc, par, ci, w):
                # a = P_hn * r (bias pre-injected; DVE — GPSIMD can't do PSUM)
                at = gp.tile([128, 1024], f32, tag=f"a{par}", name=f"a{ci}")
                nhv = nh[:].rearrange("p (s c) -> p s c", c=512)
                atv = at[:].rearrange("p (t c) -> p t c", t=2)
                rzcv = rzc[:].rearrange("p (s c) -> p s c", c=512)
                nc.vector.tensor_mul(atv[:, :, 0:w], nhv[:, 2:4, 0:w],
                                     rzcv[:, 0:2, 0:w])
                b2 = gp.tile([128, 1024], f32, tag=f"b2{par}", name=f"b2{ci}")
                nc.vector.tensor_add(
                    b2[:].rearrange("p (t c) -> p t c", t=2)[:, :, 0:w],
                    atv[:, :, 0:w], nhv[:, 0:2, 0:w])
                return b2

            def flush_pending(last=False):
                if not pending:
                    return
                rzc, b2, hg_, cstart, goff, w, par, ci = pending.pop()
                gsb = slice(goff, goff + w)
                n_t = gp.tile([128, 1024], f16, tag=f"n{par}", name=f"n{ci}")
                ntv = n_t[:].rearrange("p (t c) -> p t c", t=2)
                nc.scalar.activation(
                    ntv[:, :, 0:w],
                    b2[:].rearrange("p (t c) -> p t c", t=2)[:, :, 0:w],
                    ACT.Tanh, scale=1.0 / 32)
                d_t = gp.tile([128, 1024], f16, tag=f"d{par}", name=f"d{ci}")
                dtv = d_t[:].rearrange("p (t c) -> p t c", t=2)
                # steady-state d runs on the otherwise-idle Pool engine
                # (SBUF-only there); the final chunk keeps DVE for latency
                eng = nc.vector
                eng.tensor_sub(
                    dtv[:, :, 0:w],
                    hg_[:].rearrange("p (t b) -> p t b", t=2)[:, :, gsb],
                    ntv[:, :, 0:w])
                m_t = gp.tile([128, 1024], f16, tag=f"m{par}", name=f"m{ci}")
                mtv = m_t[:].rearrange("p (t c) -> p t c", t=2)
                nc.vector.tensor_mul(
                    mtv[:, :, 0:w],
                    rzc[:].rearrange("p (s c) -> p s c", c=512)[:, 2:4, 0:w],
                    dtv[:, :, 0:w])
                o_t = gp.tile([128, 1024], f16, tag=f"o{par}", name=f"o{ci}")
                otv = o_t[:].rearrange("p (t c) -> p t c", t=2)
                nc.vector.tensor_add(otv[:, :, 0:w], ntv[:, :, 0:w],
                                     mtv[:, :, 0:w])
                nc.sync.dma_start(
                    oT.ap()[:, cstart:cstart + w]
                        .rearrange("(t p) c -> p t c", p=128),
                    otv[:, :, 0:w])

            def chunk_tail(nh, rzc, hg_, cstart, goff, w, par, ci):
                flush_pending()
                b2 = emit_ab2(nh, rzc, par, ci, w)
                pending.append((rzc, b2, hg_, cstart, goff, w, par, ci))

            for ci, (g, goff, w) in enumerate(CHUNKS):
                if g == 0:
                    xgv, hg = g0v, h0
                else:
                    xgv = xgs[g][:].rearrange("p (k b) -> p k b", k=KIN)
                    hg = hgs[g]
                par = "AB"[ci % 2]
                cs = slice(goff, goff + w)
                cstart = g * 512 + goff
                rz = pp.tile([128, 2048], f32, tag="rz", name=f"rz{ci}")
                nh = pp.tile([128, 2048], f32, tag="nh", name=f"nh{ci}")
                rzc = gp.tile([128, 2048], f16, tag=f"rzc{par}",
                              name=f"rzc{ci}")

                if g == 0:
                    # chunk 0 is DMA-starved: z's x_lo pass (whose data
                    # arrives last) goes at the end, with the i_n x_hi wave
                    # filling the gap
                    mm_rz_xhi(rz, xgv, cs, w, tiles=(2, 3, 0, 1))
                    mm_hside_rz(rz, xgv, cs, w, tiles=(2, 3), stop=False)
                    mm_hside_rz(rz, xgv, cs, w, tiles=(0, 1))
                    sigmoid_part(rz, rzc, 0, 2, w)
                    mm_hn(nh, xgv, cs, w)
                    mm_in_xhi(nh, xgv, cs, w)
                    mm_zlo(rz, xgv, cs, w, stop=True)
                    sigmoid_part(rz, rzc, 2, 4, w)
                    mm_inlo(nh, xgv, cs, w)
                else:
                    mm_rz_xhi(rz, xgv, cs, w)
                    mm_zlo(rz, xgv, cs, w, stop=False)
                    mm_hside_rz(rz, xgv, cs, w, tiles=(2, 3))
                    mm_hside_rz(rz, xgv, cs, w, tiles=(0, 1))
                    mm_hn(nh, xgv, cs, w)
                    sigmoid_part(rz, rzc, 0, 4, w)
                    mm_in_xhi(nh, xgv, cs, w)
                    mm_inlo(nh, xgv, cs, w)
                chunk_tail(nh, rzc, hg, cstart, goff, w, par, ci)

            flush_pending(last=True)

    nc.compile()
    return nc


def _get_nc():
    global _cached
    if _cached is None:
        _cached = _build()
    return _cached


def _q8(v):
    return np.asarray(v, np.float32).astype(F8)


def kernel(input, hidden, W_ih, W_hh, b_ih, b_hh):
    input = np.asarray(input, dtype=np.float32)
    hidden = np.asarray(hidden, dtype=np.float32)
    W_ih = np.asarray(W_ih, dtype=np.float32)
    W_hh = np.asarray(W_hh, dtype=np.float32)
    b_ih = np.asarray(b_ih, dtype=np.float32)
    b_hh = np.asarray(b_hh, dtype=np.float32)

    nc = _get_nc()
    from concourse.bass_utils import run_bass_kernel_spmd

    in_maps = []
    for n in range(NUM_BLOCKS):
        WiT = 32.0 * W_ih[n].T                    # [1024, 768] fp32
        WhT = 32.0 * W_hh[n].T                    # [256, 768]
        Wi8 = _q8(WiT)
        Wh8 = _q8(WhT)
        Wi8f = Wi8.astype(np.float32)
        wpk = np.concatenate([Wi8, Wh8], axis=0)  # fp8 [1280, 768]
        E = _q8(16.0 * (WiT - Wi8f)).astype(np.float32)
        wcor = (E[:, 2 * BS:] / 16.0).astype(F8)  # [1024, 256]

        brz = b_ih[n, :2 * BS] + b_hh[n, :2 * BS]
        bin_ = b_ih[n, 2 * BS:]
        bhn = b_hh[n, 2 * BS:]
        b_vec = np.concatenate([brz, bin_]) * 32.0
        c, *_ = np.linalg.lstsq(Wi8f.T, b_vec, rcond=None)

        xc = input + c[None, :]
        x_hi = _q8(xc)
        x_lo = _q8(16.0 * (xc - x_hi.astype(np.float32)))
        hb = hidden[:, n * BS:(n + 1) * BS]
        h8 = _q8(hb)
        xin = np.concatenate(
            [np.ascontiguousarray(x_hi.T), np.ascontiguousarray(x_lo.T),
             np.ascontiguousarray(h8.T)], axis=0)  # [2304, 2048] fp8

        bim_t = np.ascontiguousarray(
            (32.0 * bhn).reshape(1, 256)).astype(F8)

        in_maps.append({
            "xin": xin,
            "wpk": np.ascontiguousarray(wpk),
            "wcor": np.ascontiguousarray(wcor),
            "bim": bim_t,
            "h16": np.ascontiguousarray(hb.T.astype(np.float16)),
        })

    res = run_bass_kernel_spmd(nc, in_maps, core_ids=list(range(NUM_BLOCKS)))
    out = np.empty((BATCH, HIDDEN_DIM), dtype=np.float32)
    for n in range(NUM_BLOCKS):
        out[:, n * BS:(n + 1) * BS] = res.results[n]["oT"].T.astype(np.float32)
    return out
